# revision 56
# baseline (speedup 1.0000x reference)
"""Multi-head self-attention Trainium2 kernel (8 NeuronCores, SPMD).

Problem: B=2, S=2048, D=1024, H=16, Dk=64; torch-style Linear projections
(x @ W.T + b), custom softmax: p = exp(scores/8), attn = p / (sum(p) + 1e-8).

Sharding: 32 (batch, head) pairs over 8 cores -> core c handles batch c//4,
heads [4*(c%4), 4*(c%4)+4). Each core projects only its 256 features of
q/k/v; attention is embarrassingly parallel over (b, h).

All matmul operands are bf16 (same PE rate as fp32r at these shapes, but
half the DMA traffic and LDWEIGHTS time; fp32 accumulate in PSUM keeps the
contractions exact). fp8 q/k via DoubleRow was measured at 2.1e-2 end-to-end
and rejected. The PE is the bottleneck engine (~155-165 us of matmul rows),
so the whole kernel is ONE continuous PE instruction stream with no phase
barriers:
  - prologue: only kT[0]/qT[0] chunk 0 (17 matmuls); everything else is
    deadline-paced filler inside the attention loop
  - 128-step attention pipeline over 8 (pair, s-chunk) blocks. Per step:
    2 score matmuls (tile_position row-groups run concurrently in the PE
    array), one exp covering both heads, 2 AV matmuls issued with a 2-step
    LAG so their exp semaphores have long fired when they reach the head
    of the in-order PE queue. Filler pops BETWEEN the exp and the AVs so
    the PE chews projections instead of stalling.
  - filler: remaining projections (kT[0] c1-3, all v, qT[0] c1-3, all of
    pair 1) with per-unit deadlines derived from first-use steps; PSUM
    x0/x1 banks rotate between accumulation chains. v s-tile pairs pack
    two 256-col accumulation groups into one 2KB PSUM bank (the second
    group starts on still-pending-zero bytes).
  - exp split across engines: 11/16 t-steps on ACT (table exp, bf16 out),
    5/16 on DVE via a one-instruction Schraudolph in the bf16 bit domain
    (bits16 = trunc(scores*A + B) via f32->u16 convert-on-write -> bitcast
    bf16; ~3% pointwise, mostly cancelled by the sum-normalization; 8.4e-3
    end-to-end vs the 2e-2 budget)
  - DMA layout: the scalar sequencer's issue backlog gates the first exp
    (DIRECT2D issues are ~0.6us each, in-order with engine ops), so qt
    chunks ride sync (chunks 1-2 as one 1024-wide transfer for 2KB
    descriptors), weights ride scalar, wv/biases ride gpsimd SWDGE
  - finalize per block: ctx [65,512] PSUM -> SBUF copies (split DVE/ACT),
    per-128-col PE transpose, [128,1] DVE reciprocal of the transposed
    denominator column, one DVE scalar_tensor_tensor out = ctx*(1/den)+bv

Output per core: [2048, 256] fp32 -> host concatenates features per batch.
"""

import sys

sys.path.insert(0, "/opt/trn_rl_repo")

from collections import deque
from contextlib import ExitStack

import ml_dtypes
import numpy as np

import concourse.bass as bass
import concourse.tile as tile
from concourse import bacc, mybir
from concourse.bass_utils import run_bass_kernel_spmd
from concourse.masks import make_identity

F32 = mybir.dt.float32
F32R = mybir.dt.float32r
BF16 = mybir.dt.bfloat16
U16 = mybir.dt.uint16

# Schraudolph exp on DVE, in the bf16 bit domain (the BIR verifier forbids
# int-typed producers feeding fp32r matmuls; bf16 has no such rule):
# exp(g/8) ~= bitcast_bf16(trunc(g*A + B)) via the f32->u16 convert-on-write.
# A = 2^7*log2(e)/8; the down-bias C=5.1 minimizes max pointwise rel err.
SCH_A = float(np.float32((1 << 7) / (8.0 * np.log(2.0))))
SCH_B = float(np.float32((127 << 7) - 5.1))
# t-steps per 16-step block whose exp runs on DVE (rest on ACT)
SCH_STEPS = frozenset({2, 5, 8, 11, 14})

S = 2048  # sequence length
D = 1024  # d_model
J = 256  # features per core (4 heads x 64)
NKT = 8  # k-tiles of the d_model contraction
NSC = 4  # s-chunks of 512
NTT = 16  # t-tiles of 128
N_CORES = 8

_cached_nc = None
last_result = None  # BassKernelResults of the most recent run (for test.py)


def _round_fp32r(x: np.ndarray) -> np.ndarray:
    """Round fp32 to fp32r (keep 11 mantissa bits, round to nearest even)."""
    u = np.ascontiguousarray(x, dtype=np.float32).view(np.uint32)
    r = (u.astype(np.uint64) + 0x7FF + ((u >> 12) & 1)) & 0xFFFFF000
    return r.astype(np.uint32).view(np.float32)


def _build():
    nc = bacc.Bacc(None, target_bir_lowering=False)

    qt = nc.dram_tensor("qt", [D, S], BF16, kind="ExternalInput")
    wq = nc.dram_tensor("wq", [D, J], BF16, kind="ExternalInput")
    wk = nc.dram_tensor("wk", [D, J], BF16, kind="ExternalInput")
    wv = nc.dram_tensor("wv", [D, J], BF16, kind="ExternalInput")
    bq = nc.dram_tensor("bq", [J], F32, kind="ExternalInput")
    bk = nc.dram_tensor("bk", [J], F32, kind="ExternalInput")
    bv = nc.dram_tensor("bv", [J], F32, kind="ExternalInput")
    out = nc.dram_tensor("out", [S, J], F32, kind="ExternalOutput")

    with tile.TileContext(nc) as tc, ExitStack() as ctx:
        wts = ctx.enter_context(tc.tile_pool(name="wts", bufs=1))
        qkp = ctx.enter_context(tc.tile_pool(name="qkp", bufs=1))
        vxp = ctx.enter_context(tc.tile_pool(name="vxp", bufs=1))
        bp = ctx.enter_context(tc.tile_pool(name="bp", bufs=1))
        cxp = ctx.enter_context(tc.tile_pool(name="cxp", bufs=12))
        pTp = ctx.enter_context(tc.tile_pool(name="pTp", bufs=4))
        outp = ctx.enter_context(tc.tile_pool(name="outp", bufs=1))
        qtcp = ctx.enter_context(tc.tile_pool(name="qtc", bufs=1))
        rp = ctx.enter_context(tc.tile_pool(name="rp", bufs=8))
        aps = ctx.enter_context(tc.tile_pool(name="aps", bufs=1, space="PSUM"))
        p1b = ctx.enter_context(tc.tile_pool(name="p1b", bufs=1, space="PSUM"))

        # Weights: 8 k-tiles each of [128, 256], k-major and split across the
        # HWDGE (sync/scalar) / SWDGE (gpsimd) queues, interleaved with the
        # first s-chunk of QT so the first projection matmuls start early
        wq_t = [
            wts.tile([128, J], BF16, name=f"wq{k}", tag=f"wq{k}") for k in range(NKT)
        ]
        wk_t = [
            wts.tile([128, J], BF16, name=f"wk{k}", tag=f"wk{k}") for k in range(NKT)
        ]
        wv_t = [
            wts.tile([128, J], BF16, name=f"wv{k}", tag=f"wv{k}") for k in range(NKT)
        ]
        qtc = [
            qtcp.tile([128, NKT, 512], BF16, name=f"qtc{c}", tag=f"qtc{c}")
            for c in range(NSC)
        ]

        def qtc_ap(c, k, js=None):
            base = qtc[c][:, k, :]
            return base if js is None else base[:, js]
        # ident is built first on gpsimd, ahead of its DMA-issue backlog
        ident = bp.tile([128, 128], F32, name="ident")
        make_identity(nc, ident[:])

        # ALL qtc issues on sync: the scalar sequencer must stay shallow --
        # its DMA-issue backlog (0.6us each, in-order with engine ops) would
        # otherwise gate the first exp
        for k in range(NKT):
            ksl = slice(k * 128, (k + 1) * 128)
            nc.sync.dma_start(qtc[0][:, k, :], qt[ksl, 0:512])
            nc.scalar.dma_start(wk_t[k][:], wk[ksl, :])
        for k in range(NKT):
            ksl = slice(k * 128, (k + 1) * 128)
            nc.scalar.dma_start(wq_t[k][:], wq[ksl, :])
            nc.gpsimd.dma_start(wv_t[k][:], wv[ksl, :])
        # chunk 1 split across sync+scalar (needed by step ~3); chunk 3 on
        # sync behind it; chunk 2 rides gpsimd SWDGE (issued after wv below)
        for k in range(NKT):
            eng = nc.sync if k % 2 == 0 else nc.scalar
            eng.dma_start(qtc[1][:, k, :], qt[k * 128 : (k + 1) * 128, 512:1024])
        for k in range(NKT):
            nc.sync.dma_start(qtc[3][:, k, :], qt[k * 128 : (k + 1) * 128, 1536:2048])

        # Biases: bq/bk as per-partition scalars [128, 2]; bv broadcast [128, 256]
        bq_t = bp.tile([128, 2], F32, name="bqt")
        nc.gpsimd.dma_start(bq_t[:], bq.rearrange("(m p) -> p m", p=128))
        bk_t = bp.tile([128, 2], F32, name="bkt")
        nc.gpsimd.dma_start(bk_t[:], bk.rearrange("(m p) -> p m", p=128))
        bv_t = bp.tile([128, J], F32, name="bvt")
        bvap = bv[:]
        bv_bcast = bass.AP(
            tensor=bvap.tensor, offset=bvap.offset, ap=[[0, 128], [1, J]]
        )
        nc.gpsimd.dma_start(bv_t[:], bv_bcast)
        for k in range(NKT):
            nc.gpsimd.dma_start(
                qtc[2][:, k, :], qt[k * 128 : (k + 1) * 128, 1024:1536]
            )

        scratch = bp.tile([128, 1], F32, name="scratch")

        # Persistent projected tensors
        qT = [qkp.tile([128, S], BF16, name=f"qT{m}", tag=f"qT{m}") for m in range(2)]
        kT = [qkp.tile([128, S], BF16, name=f"kT{m}", tag=f"kT{m}") for m in range(2)]
        v_ext = []
        for t in range(NTT):
            vt = vxp.tile([128, 4, 65], BF16, name=f"vx{t}", tag=f"vx{t}")
            # DVE memsets (it is idle during the prologue; gpsimd's queue is
            # busy streaming wv and would delay the first v epilogues)
            nc.vector.memset(vt[:], 1.0)  # ones col [:, h, 64] survives
            v_ext.append(vt)
        out_tiles = [
            outp.tile([128, J], F32, name=f"ot{b}", tag=f"ot{b}") for b in range(16)
        ]

        # pre-load the ACT exp table (it only needs bq_t) so the first
        # attention exp doesn't pay the ~2.7us table-load stall
        nc.scalar.activation(
            scratch[:], bq_t[:, 0:1], mybir.ActivationFunctionType.Exp, scale=0.0
        )

        # ---------- projection instruction chains ----------
        xrot = [0]

        def xtile(shape=(128, 512)):
            t_ = p1b.tile(list(shape), F32, name="px", tag=f"x{xrot[0] % 2}")
            xrot[0] += 1
            return t_

        def emit_qk_chain(pair, c, which, korder=None):
            """8 matmuls + bias epilogue for qT/kT[pair][:, c*512:(c+1)*512]."""
            st = {}
            w_t = wq_t if which == "q" else wk_t
            dst = (qT if which == "q" else kT)[pair]
            b_t = bq_t if which == "q" else bk_t
            ks = list(range(NKT)) if korder is None else korder

            def mm(k):
                def f():
                    if k == ks[0]:
                        st["t"] = xtile()
                    nc.tensor.matmul(
                        st["t"][:],
                        w_t[k][:, pair * 128 : (pair + 1) * 128],
                        qtc_ap(c, k),
                        start=(k == ks[0]),
                        stop=(k == ks[-1]),
                    )
                return f

            def epi():
                s0 = c * 512
                nc.vector.tensor_scalar_add(
                    dst[:, s0 : s0 + 512], st.pop("t")[:], b_t[:, pair : pair + 1]
                )

            return [mm(k) for k in ks] + [epi]

        def emit_pv_chain(c, half):
            """v for s-tiles (4c+2*half, 4c+2*half+1): 16 matmuls packed into
            one [128,512] PSUM bank (two 256-col accumulation groups; the
            second group starts on still-pending-zero bytes), + 2 epilogues."""
            st = {}
            thunks = []

            def mm(k, sub):
                def f():
                    if k == 0 and sub == 0:
                        st["t"] = xtile()
                    i = 2 * half + sub
                    nc.tensor.matmul(
                        st["t"][:, sub * 256 : (sub + 1) * 256],
                        qtc_ap(c, k, slice(i * 128, (i + 1) * 128)),
                        wv_t[k][:],
                        start=(k == 0 and sub == 0),
                        stop=(k == NKT - 1 and sub == 1),
                        skip_group_check=True,
                    )
                return f

            for sub in range(2):
                for k in range(NKT):
                    thunks.append(mm(k, sub))

            def epi(sub):
                def f():
                    i = 2 * half + sub
                    src = st["t"][:, sub * 256 : (sub + 1) * 256]
                    nc.vector.tensor_copy(
                        v_ext[c * 4 + i][:, :, 0:64],
                        src.rearrange("p (h d) -> p h d", h=4),
                    )
                    if sub == 1:
                        st.pop("t")
                return f

            thunks += [epi(0), epi(1)]
            return thunks

        # ---------- prologue: minimum prefix (kT[0]/qT[0] chunk 0) ----------
        for th in emit_qk_chain(0, 0, "k") + emit_qk_chain(0, 0, "q"):
            th()

        # ---------- filler: remaining projections, deadline-paced ----------
        chains = [
            (emit_pv_chain(0, 0), 0),
            (emit_pv_chain(0, 1), 1),
            (emit_qk_chain(0, 1, "k"), 3),
            (emit_pv_chain(1, 0), 4),
            (emit_pv_chain(1, 1), 5),
            (emit_qk_chain(0, 2, "k"), 7),
            (emit_pv_chain(2, 0), 8),
            (emit_pv_chain(2, 1), 9),
            (emit_qk_chain(0, 3, "k"), 11),
            (emit_pv_chain(3, 0), 12),
            (emit_pv_chain(3, 1), 13),
            (emit_qk_chain(0, 1, "q"), 15),
            (emit_qk_chain(0, 2, "q"), 30),
            (emit_qk_chain(0, 3, "q"), 42),
            (emit_qk_chain(1, 0, "k"), 48),
            (emit_qk_chain(1, 0, "q"), 54),
            (emit_qk_chain(1, 1, "k"), 60),
            (emit_qk_chain(1, 1, "q"), 64),
            (emit_qk_chain(1, 2, "k"), 68),
            (emit_qk_chain(1, 2, "q"), 71),
            (emit_qk_chain(1, 3, "k"), 74),
            (emit_qk_chain(1, 3, "q"), 77),
        ]
        work = deque()
        prev_dl = 0
        for thunks, dl in chains:
            n = len(thunks)
            for j, th in enumerate(thunks):
                work.append((prev_dl + (dl - prev_dl) * (j + 1) / n, th))
            prev_dl = dl

        # ---------- attention: one continuous 128-step pipeline ----------
        pieces = deque()
        done_cnt = {}
        piece_tags = [("x0", "x1")]

        def piece(cs_tile, sc, h, pi):
            def f():
                tags = piece_tags[0]
                tag = tags[xrot[0] % len(tags)]
                xrot[0] += 1
                if tag.startswith("x"):
                    tp = p1b.tile([128, 65], F32, name="tp", tag=tag)
                else:
                    tp = aps.tile([128, 65], F32, name="tp", tag=tag, bufs=2)
                nc.tensor.transpose(
                    tp[:],
                    cs_tile[0:65, pi * 128 : (pi + 1) * 128],
                    ident[0:65, 0:65],
                )
                blk = sc * 4 + pi
                r = rp.tile([128, 1], F32, name="r", tag="r")
                nc.vector.reciprocal(r[:], tp[:, 64:65])
                nc.vector.scalar_tensor_tensor(
                    out=out_tiles[blk][:, h * 64 : (h + 1) * 64],
                    in0=tp[:, 0:64],
                    scalar=r[:],
                    in1=bv_t[:, h * 64 : (h + 1) * 64],
                    op0=mybir.AluOpType.mult,
                    op1=mybir.AluOpType.add,
                )
                done_cnt[blk] = done_cnt.get(blk, 0) + 1
                if done_cnt[blk] == 4:
                    eng = nc.sync if blk % 2 == 0 else nc.scalar
                    eng.dma_start(
                        out[blk * 128 : (blk + 1) * 128, :], out_tiles[blk][:]
                    )
            return f

        def fill_slot(i):
            popped = False
            while work and work[0][0] <= i:
                work.popleft()[1]()
                popped = True
            if pieces and not work and not popped:
                pieces.popleft()()
                if len(pieces) > 16 and pieces:
                    pieces.popleft()()

        blocks = [(p, sc) for p in range(2) for sc in range(NSC)]
        NB = len(blocks)
        LAG = 2  # AV issued 2 steps behind its exp: the sem has long fired,
        #          so the in-order PE never stalls at the AV queue head
        ctx_ps = {}
        pts = {}
        for i in range(NB * NTT + LAG):
            if i < NB * NTT:
                b, t = divmod(i, NTT)
                pair, sc = blocks[b]
                s0 = sc * 512
                qTt, kTt = qT[pair], kT[pair]
                if t == 0:
                    ctxA = aps.tile([65, 512], F32, name="ctxA", tag="ctx", bufs=2)
                    ctxB = aps.tile([65, 512], F32, name="ctxB", tag="ctx", bufs=2)
                    ctx_ps[b] = (ctxA, ctxB)
                tsl = slice(t * 128, (t + 1) * 128)
                # both heads' scoresT share one 2-bank tile so a single
                # exp instruction covers both
                g = aps.tile([128, 1024], F32, name="g", tag="grp", bufs=2)
                nc.tensor.matmul(
                    g[:, 0:512],
                    kTt[0:64, tsl],
                    qTt[0:64, s0 : s0 + 512],
                    start=True,
                    stop=True,
                    tile_position=(0, 0),
                )
                nc.tensor.matmul(
                    g[:, 512:1024],
                    kTt[64:128, tsl],
                    qTt[64:128, s0 : s0 + 512],
                    start=True,
                    stop=True,
                    tile_position=(64, 0),
                )
                pT_ = pTp.tile([128, 1024], BF16, name="pT_", tag="pT")
                if t in SCH_STEPS:
                    nc.vector.tensor_scalar(
                        out=pT_[:].bitcast(U16),
                        in0=g[:],
                        scalar1=SCH_A,
                        scalar2=SCH_B,
                        op0=mybir.AluOpType.mult,
                        op1=mybir.AluOpType.add,
                    )
                else:
                    nc.scalar.activation(
                        pT_[:], g[:],
                        mybir.ActivationFunctionType.Exp, scale=0.125,
                    )
                pts[i] = pT_
            # filler between exp-issue and the AV that consumes the previous
            # exp: the PE chews projections/transposes instead of stalling at
            # the queue head while ACT finishes
            fill_slot(i)
            if i >= LAG:
                b, t = divmod(i - LAG, NTT)
                pair, sc = blocks[b]
                hA, hB = 2 * pair, 2 * pair + 1
                ctxA, ctxB = ctx_ps[b]
                pT_ = pts.pop(i - LAG)
                st_, sp_ = (t == 0), (t == NTT - 1)
                nc.tensor.matmul(
                    ctxA[:], v_ext[t][:, hA, :], pT_[:, 0:512],
                    start=st_, stop=sp_,
                )
                nc.tensor.matmul(
                    ctxB[:], v_ext[t][:, hB, :], pT_[:, 512:1024],
                    start=st_, stop=sp_,
                )
                if t == NTT - 1:
                    del ctx_ps[b]
                    csA = cxp.tile([65, 512], F32, name="csA", tag="cs")
                    nc.vector.tensor_copy(csA[:], ctxA[:])
                    csB = cxp.tile([65, 512], F32, name="csB", tag="cs")
                    nc.scalar.copy(csB[:], ctxB[:])
                    for pi in range(4):
                        pieces.append(piece(csA, sc, hA, pi))
                        pieces.append(piece(csB, sc, hB, pi))

        # drain
        while work:
            work.popleft()[1]()
        piece_tags[0] = ("x0", "x1", "grp", "grp")
        while pieces:
            pieces.popleft()()

    nc.compile()
    return nc


def kernel(Q, Wq, bq, Wk, bk, Wv, bv):
    global _cached_nc, last_result
    Q = np.asarray(Q, dtype=np.float32)
    Wq, Wk, Wv = (np.asarray(w, dtype=np.float32) for w in (Wq, Wk, Wv))
    bq, bk, bv = (np.asarray(b, dtype=np.float32) for b in (bq, bk, bv))
    B = Q.shape[0]
    assert Q.shape == (B, S, D) and B * 4 == N_CORES

    if _cached_nc is None:
        _cached_nc = _build()
    nc = _cached_nc

    # host-side shard prep (bf16 inputs: full PE rate, half the DMA traffic)
    bf16 = ml_dtypes.bfloat16
    qts = [np.ascontiguousarray(Q[b].T).astype(bf16) for b in range(B)]
    wqs = [np.ascontiguousarray(Wq[g * J : (g + 1) * J, :].T).astype(bf16) for g in range(4)]
    wks = [np.ascontiguousarray(Wk[g * J : (g + 1) * J, :].T).astype(bf16) for g in range(4)]
    wvs = [np.ascontiguousarray(Wv[g * J : (g + 1) * J, :].T).astype(bf16) for g in range(4)]

    in_maps = []
    for c in range(N_CORES):
        b, g = c // 4, c % 4
        jsl = slice(g * J, (g + 1) * J)
        in_maps.append(
            {
                "qt": qts[b],
                "wq": wqs[g],
                "wk": wks[g],
                "wv": wvs[g],
                "bq": np.ascontiguousarray(bq[jsl]),
                "bk": np.ascontiguousarray(bk[jsl]),
                "bv": np.ascontiguousarray(bv[jsl]),
            }
        )

    last_result = run_bass_kernel_spmd(nc, in_maps, list(range(N_CORES)))

    full = np.empty((B, S, D), dtype=np.float32)
    for c in range(N_CORES):
        b, g = c // 4, c % 4
        full[b, :, g * J : (g + 1) * J] = last_result.results[c]["out"]
    return full


# revision 57
# speedup vs baseline: 1.0166x; 1.0166x over previous
"""Multi-head self-attention Trainium2 kernel (8 NeuronCores, SPMD).

Problem: B=2, S=2048, D=1024, H=16, Dk=64; torch-style Linear projections
(x @ W.T + b), custom softmax: p = exp(scores/8), attn = p / (sum(p) + 1e-8).

Sharding: 32 (batch, head) pairs over 8 cores -> core c handles batch c//4,
heads [4*(c%4), 4*(c%4)+4). Each core projects only its 256 features of
q/k/v; attention is embarrassingly parallel over (b, h).

Per-core kernel: matmuls in fp32r (fp32 with 11 mantissa bits, full PE rate
at moving-dim >= 256). The PE is the bottleneck engine (~165 us of matmul
rows), so the whole kernel is ONE continuous PE stream with no phase
barriers:
  - prologue: kT[0]/v for s-chunks 0-1 and qT[0] chunk 0 (PE ramps while
    the remaining qt chunks DMA in)
  - 128-step attention pipeline over 8 (pair, s-chunk) blocks; the
    remaining projections (kT[0] c2-3, v c2-3, qT[0] c1-3, all of pair 1)
    run as deadline-paced filler between steps, keeping the PE dense so
    the HAM clock gate never rethrottles it
  - exp is split across engines: most t-steps on ACT (table exp, bf16 out),
    5/16 on DVE via a one-instruction Schraudolph in the bf16 bit domain
    (bits16 = trunc(scores*A + B) -> bitcast bf16; ~3% pointwise, mostly
    cancelled by the sum-normalization; ~8e-3 end-to-end vs 2e-2 budget)
  - v and p are bf16 (matmul rate is identical, halves the exp-side SBUF
    and removes the fp32r-producer constraint); scores/q/k stay fp32r --
    fp8 q/k was measured at 2.1e-2 end-to-end and rejected
  - finalize per block: ctx [65,512] PSUM -> SBUF copy (ACT), reciprocal
    in-place on the denominator row (DVE), then per-128-col PE transpose
    and one DVE scalar_tensor_tensor out = ctx*(1/den) + bv reading the
    reciprocal straight from PSUM col 64

Output per core: [2048, 256] fp32 -> host concatenates features per batch.
"""

import sys

sys.path.insert(0, "/opt/trn_rl_repo")

from collections import deque
from contextlib import ExitStack

import ml_dtypes
import numpy as np

import concourse.bass as bass
import concourse.tile as tile
from concourse import bacc, mybir
from concourse.bass_utils import run_bass_kernel_spmd
from concourse.masks import make_identity

F32 = mybir.dt.float32
F32R = mybir.dt.float32r
BF16 = mybir.dt.bfloat16
U16 = mybir.dt.uint16

# Schraudolph exp on DVE, in the bf16 bit domain (the BIR verifier forbids
# int-typed producers feeding fp32r matmuls; bf16 has no such rule):
# exp(g/8) ~= bitcast_bf16(trunc(g*A + B)) via the f32->u16 convert-on-write.
# A = 2^7*log2(e)/8; the down-bias C=5.1 minimizes max pointwise rel err.
SCH_A = float(np.float32((1 << 7) / (8.0 * np.log(2.0))))
SCH_B = float(np.float32((127 << 7) - 5.1))
# t-steps per 16-step block whose exp runs on DVE (rest on ACT)
SCH_STEPS = frozenset({2, 5, 8, 11, 14})

S = 2048  # sequence length
D = 1024  # d_model
J = 256  # features per core (4 heads x 64)
NKT = 8  # k-tiles of the d_model contraction
NSC = 4  # s-chunks of 512
NTT = 16  # t-tiles of 128
N_CORES = 8

_cached_nc = None
last_result = None  # BassKernelResults of the most recent run (for test.py)


def _round_fp32r(x: np.ndarray) -> np.ndarray:
    """Round fp32 to fp32r (keep 11 mantissa bits, round to nearest even)."""
    u = np.ascontiguousarray(x, dtype=np.float32).view(np.uint32)
    r = (u.astype(np.uint64) + 0x7FF + ((u >> 12) & 1)) & 0xFFFFF000
    return r.astype(np.uint32).view(np.float32)


def _build():
    nc = bacc.Bacc(None, target_bir_lowering=False)

    qt = nc.dram_tensor("qt", [D, S], BF16, kind="ExternalInput")
    wq = nc.dram_tensor("wq", [D, J], BF16, kind="ExternalInput")
    wk = nc.dram_tensor("wk", [D, J], BF16, kind="ExternalInput")
    wv = nc.dram_tensor("wv", [D, J], BF16, kind="ExternalInput")
    bq = nc.dram_tensor("bq", [J], F32, kind="ExternalInput")
    bk = nc.dram_tensor("bk", [J], F32, kind="ExternalInput")
    bv = nc.dram_tensor("bv", [J], F32, kind="ExternalInput")
    out = nc.dram_tensor("out", [S, J], F32, kind="ExternalOutput")

    with tile.TileContext(nc) as tc, ExitStack() as ctx:
        wts = ctx.enter_context(tc.tile_pool(name="wts", bufs=1))
        qkp = ctx.enter_context(tc.tile_pool(name="qkp", bufs=1))
        vxp = ctx.enter_context(tc.tile_pool(name="vxp", bufs=1))
        bp = ctx.enter_context(tc.tile_pool(name="bp", bufs=1))
        cxp = ctx.enter_context(tc.tile_pool(name="cxp", bufs=12))
        pTp = ctx.enter_context(tc.tile_pool(name="pTp", bufs=4))
        outp = ctx.enter_context(tc.tile_pool(name="outp", bufs=1))
        qtcp = ctx.enter_context(tc.tile_pool(name="qtc", bufs=1))
        rp = ctx.enter_context(tc.tile_pool(name="rp", bufs=8))
        aps = ctx.enter_context(tc.tile_pool(name="aps", bufs=1, space="PSUM"))
        p1b = ctx.enter_context(tc.tile_pool(name="p1b", bufs=1, space="PSUM"))

        # Weights: 8 k-tiles each of [128, 256], k-major and split across the
        # HWDGE (sync/scalar) / SWDGE (gpsimd) queues, interleaved with the
        # first s-chunk of QT so the first projection matmuls start early
        wq_t = [
            wts.tile([128, J], BF16, name=f"wq{k}", tag=f"wq{k}") for k in range(NKT)
        ]
        wk_t = [
            wts.tile([128, J], BF16, name=f"wk{k}", tag=f"wk{k}") for k in range(NKT)
        ]
        wv_t = [
            wts.tile([128, J], BF16, name=f"wv{k}", tag=f"wv{k}") for k in range(NKT)
        ]
        qtc = [
            qtcp.tile([128, NKT, 512], BF16, name=f"qtc{c}", tag=f"qtc{c}")
            for c in range(NSC)
        ]
        # qtc chunk k-tiles alternate sync/vector HWDGE queues so the first
        # k-tiles land fast; wk streams ahead of wq on scalar (the k chain
        # runs first); wv + biases go via gpsimd SWDGE
        # ident is built FIRST on gpsimd (before its DMA-issue backlog) so
        # the PE warm-up transposes can start at ~7us
        ident = bp.tile([128, 128], F32, name="ident")
        make_identity(nc, ident[:])

        # ALL qtc issues on sync: the scalar sequencer must stay shallow --
        # its DMA-issue backlog (0.6us each, in-order with engine ops) would
        # otherwise gate the first exp
        for k in range(NKT):
            ksl = slice(k * 128, (k + 1) * 128)
            nc.sync.dma_start(qtc[0][:, k, :], qt[ksl, 0:512])
            nc.scalar.dma_start(wk_t[k][:], wk[ksl, :])
        for k in range(NKT):
            ksl = slice(k * 128, (k + 1) * 128)
            nc.scalar.dma_start(wq_t[k][:], wq[ksl, :])
            nc.gpsimd.dma_start(wv_t[k][:], wv[ksl, :])
        # chunk 1 is needed soonest (attention t=4..7 plus v s-tiles 4-7):
        # split it across both queues; chunks 2-3 follow on sync
        for k in range(NKT):
            eng = nc.sync if k % 2 == 0 else nc.scalar
            eng.dma_start(qtc[1][:, k, :], qt[k * 128 : (k + 1) * 128, 512:1024])
        for c in range(2, NSC):
            s0 = c * 512
            for k in range(NKT):
                nc.sync.dma_start(
                    qtc[c][:, k, :], qt[k * 128 : (k + 1) * 128, s0 : s0 + 512]
                )

        # Biases: bq/bk as per-partition scalars [128, 2]; bv broadcast [128, 256]
        bq_t = bp.tile([128, 2], F32, name="bqt")
        nc.gpsimd.dma_start(bq_t[:], bq.rearrange("(m p) -> p m", p=128))
        bk_t = bp.tile([128, 2], F32, name="bkt")
        nc.gpsimd.dma_start(bk_t[:], bk.rearrange("(m p) -> p m", p=128))
        bv_t = bp.tile([128, J], F32, name="bvt")
        bvap = bv[:]
        bv_bcast = bass.AP(
            tensor=bvap.tensor, offset=bvap.offset, ap=[[0, 128], [1, J]]
        )
        nc.gpsimd.dma_start(bv_t[:], bv_bcast)

        scratch = bp.tile([128, 1], F32, name="scratch")

        # Persistent projected tensors
        qT = [qkp.tile([128, S], BF16, name=f"qT{m}", tag=f"qT{m}") for m in range(2)]
        kT = [qkp.tile([128, S], BF16, name=f"kT{m}", tag=f"kT{m}") for m in range(2)]
        v_ext = []
        for t in range(NTT):
            vt = vxp.tile([128, 4, 65], BF16, name=f"vx{t}", tag=f"vx{t}")
            # DVE memsets (it is idle during the prologue; gpsimd's queue is
            # busy streaming wv and would delay the first v epilogues)
            nc.vector.memset(vt[:], 1.0)  # ones col [:, h, 64] survives
            v_ext.append(vt)
        out_tiles = [
            outp.tile([128, J], F32, name=f"ot{b}", tag=f"ot{b}") for b in range(16)
        ]

        # pre-load the ACT exp table (it only needs bq_t) so the first
        # attention exp doesn't pay the ~2.7us table-load stall
        nc.scalar.activation(
            scratch[:], bq_t[:, 0:1], mybir.ActivationFunctionType.Exp, scale=0.0
        )

        # ---------- projection instruction chains ----------
        xrot = [0]

        def xtile(shape=(128, 512)):
            t_ = p1b.tile(list(shape), F32, name="px", tag=f"x{xrot[0] % 2}")
            xrot[0] += 1
            return t_

        def emit_qk_chain(pair, c, which, korder=None):
            """8 matmuls + bias epilogue for qT/kT[pair][:, c*512:(c+1)*512]."""
            st = {}
            w_t = wq_t if which == "q" else wk_t
            dst = (qT if which == "q" else kT)[pair]
            b_t = bq_t if which == "q" else bk_t
            ks = list(range(NKT)) if korder is None else korder

            def mm(k):
                def f():
                    if k == ks[0]:
                        st["t"] = xtile()
                    nc.tensor.matmul(
                        st["t"][:],
                        w_t[k][:, pair * 128 : (pair + 1) * 128],
                        qtc[c][:, k, :],
                        start=(k == ks[0]),
                        stop=(k == ks[-1]),
                    )
                return f

            def epi():
                s0 = c * 512
                nc.vector.tensor_scalar_add(
                    dst[:, s0 : s0 + 512], st.pop("t")[:], b_t[:, pair : pair + 1]
                )

            return [mm(k) for k in ks] + [epi]

        def emit_pv_chain(c, half):
            """v for s-tiles (4c+2*half, 4c+2*half+1): 16 matmuls packed into
            one [128,512] PSUM bank (two 256-col accumulation groups; the
            second group starts on still-pending-zero bytes), + 2 epilogues."""
            st = {}
            thunks = []

            def mm(k, sub):
                def f():
                    if k == 0 and sub == 0:
                        st["t"] = xtile()
                    i = 2 * half + sub
                    nc.tensor.matmul(
                        st["t"][:, sub * 256 : (sub + 1) * 256],
                        qtc[c][:, k, i * 128 : (i + 1) * 128],
                        wv_t[k][:],
                        start=(k == 0 and sub == 0),
                        stop=(k == NKT - 1 and sub == 1),
                        skip_group_check=True,
                    )
                return f

            for sub in range(2):
                for k in range(NKT):
                    thunks.append(mm(k, sub))

            def epi(sub):
                def f():
                    i = 2 * half + sub
                    src = st["t"][:, sub * 256 : (sub + 1) * 256]
                    nc.vector.tensor_copy(
                        v_ext[c * 4 + i][:, :, 0:64],
                        src.rearrange("p (h d) -> p h d", h=4),
                    )
                    if sub == 1:
                        st.pop("t")
                return f

            thunks += [epi(0), epi(1)]
            return thunks

        # ---------- prologue: minimum prefix (kT[0]/qT[0] chunk 0) ----------
        for th in emit_qk_chain(0, 0, "k") + emit_qk_chain(0, 0, "q"):
            th()

        # ---------- filler: remaining projections, deadline-paced ----------
        chains = [
            (emit_pv_chain(0, 0), 0),
            (emit_pv_chain(0, 1), 1),
            (emit_qk_chain(0, 1, "k"), 3),
            (emit_pv_chain(1, 0), 4),
            (emit_pv_chain(1, 1), 5),
            (emit_qk_chain(0, 2, "k"), 7),
            (emit_pv_chain(2, 0), 8),
            (emit_pv_chain(2, 1), 9),
            (emit_qk_chain(0, 3, "k"), 11),
            (emit_pv_chain(3, 0), 12),
            (emit_pv_chain(3, 1), 13),
            (emit_qk_chain(0, 1, "q"), 15),
            (emit_qk_chain(0, 2, "q"), 30),
            (emit_qk_chain(0, 3, "q"), 42),
            (emit_qk_chain(1, 0, "k"), 48),
            (emit_qk_chain(1, 0, "q"), 54),
            (emit_qk_chain(1, 1, "k"), 60),
            (emit_qk_chain(1, 1, "q"), 64),
            (emit_qk_chain(1, 2, "k"), 68),
            (emit_qk_chain(1, 2, "q"), 71),
            (emit_qk_chain(1, 3, "k"), 74),
            (emit_qk_chain(1, 3, "q"), 77),
        ]
        work = deque()
        prev_dl = 0
        for thunks, dl in chains:
            n = len(thunks)
            for j, th in enumerate(thunks):
                work.append((prev_dl + (dl - prev_dl) * (j + 1) / n, th))
            prev_dl = dl

        # ---------- attention: one continuous 128-step pipeline ----------
        pieces = deque()
        done_cnt = {}
        piece_tags = [("x0", "x1")]

        def piece(cs_tile, sc, h, pi):
            def f():
                tags = piece_tags[0]
                tag = tags[xrot[0] % len(tags)]
                xrot[0] += 1
                if tag.startswith("x"):
                    tp = p1b.tile([128, 65], F32, name="tp", tag=tag)
                else:
                    tp = aps.tile([128, 65], F32, name="tp", tag=tag, bufs=2)
                nc.tensor.transpose(
                    tp[:],
                    cs_tile[0:65, pi * 128 : (pi + 1) * 128],
                    ident[0:65, 0:65],
                )
                blk = sc * 4 + pi
                r = rp.tile([128, 1], F32, name="r", tag="r")
                nc.vector.reciprocal(r[:], tp[:, 64:65])
                nc.vector.scalar_tensor_tensor(
                    out=out_tiles[blk][:, h * 64 : (h + 1) * 64],
                    in0=tp[:, 0:64],
                    scalar=r[:],
                    in1=bv_t[:, h * 64 : (h + 1) * 64],
                    op0=mybir.AluOpType.mult,
                    op1=mybir.AluOpType.add,
                )
                key = (blk, h // 2)
                done_cnt[key] = done_cnt.get(key, 0) + 1
                if done_cnt[key] == 2:
                    jsl = slice((h // 2) * 128, (h // 2) * 128 + 128)
                    eng = nc.sync if blk % 2 == 0 else nc.scalar
                    eng.dma_start(
                        out[blk * 128 : (blk + 1) * 128, jsl],
                        out_tiles[blk][:, jsl],
                    )
            return f

        def fill_slot(i):
            popped = False
            while work and work[0][0] <= i:
                work.popleft()[1]()
                popped = True
            if pieces and not work and not popped:
                pieces.popleft()()
                if len(pieces) > 16 and pieces:
                    pieces.popleft()()

        blocks = [(p, sc) for p in range(2) for sc in range(NSC)]
        NB = len(blocks)
        LAG = 2  # AV issued 2 steps behind its exp: the sem has long fired,
        #          so the in-order PE never stalls at the AV queue head
        ctx_ps = {}
        pts = {}
        for i in range(NB * NTT + LAG):
            if i < NB * NTT:
                b, t = divmod(i, NTT)
                pair, sc = blocks[b]
                s0 = sc * 512
                qTt, kTt = qT[pair], kT[pair]
                if t == 0:
                    ctxA = aps.tile([65, 512], F32, name="ctxA", tag="ctx", bufs=2)
                    ctxB = aps.tile([65, 512], F32, name="ctxB", tag="ctx", bufs=2)
                    ctx_ps[b] = (ctxA, ctxB)
                tsl = slice(t * 128, (t + 1) * 128)
                # both heads' scoresT share one 2-bank tile so a single
                # exp instruction covers both
                g = aps.tile([128, 1024], F32, name="g", tag="grp", bufs=2)
                nc.tensor.matmul(
                    g[:, 0:512],
                    kTt[0:64, tsl],
                    qTt[0:64, s0 : s0 + 512],
                    start=True,
                    stop=True,
                    tile_position=(0, 0),
                )
                nc.tensor.matmul(
                    g[:, 512:1024],
                    kTt[64:128, tsl],
                    qTt[64:128, s0 : s0 + 512],
                    start=True,
                    stop=True,
                    tile_position=(64, 0),
                )
                pT_ = pTp.tile([128, 1024], BF16, name="pT_", tag="pT")
                if t in SCH_STEPS:
                    nc.vector.tensor_scalar(
                        out=pT_[:].bitcast(U16),
                        in0=g[:],
                        scalar1=SCH_A,
                        scalar2=SCH_B,
                        op0=mybir.AluOpType.mult,
                        op1=mybir.AluOpType.add,
                    )
                else:
                    nc.scalar.activation(
                        pT_[:], g[:],
                        mybir.ActivationFunctionType.Exp, scale=0.125,
                    )
                pts[i] = pT_
            # filler between exp-issue and the AV that consumes the previous
            # exp: the PE chews projections/transposes instead of stalling at
            # the queue head while ACT finishes
            fill_slot(i)
            if i >= LAG:
                b, t = divmod(i - LAG, NTT)
                pair, sc = blocks[b]
                hA, hB = 2 * pair, 2 * pair + 1
                ctxA, ctxB = ctx_ps[b]
                pT_ = pts.pop(i - LAG)
                st_, sp_ = (t == 0), (t == NTT - 1)
                nc.tensor.matmul(
                    ctxA[:], v_ext[t][:, hA, :], pT_[:, 0:512],
                    start=st_, stop=sp_,
                )
                nc.tensor.matmul(
                    ctxB[:], v_ext[t][:, hB, :], pT_[:, 512:1024],
                    start=st_, stop=sp_,
                )
                if t == NTT - 1:
                    del ctx_ps[b]
                    csA = cxp.tile([65, 512], F32, name="csA", tag="cs")
                    nc.vector.tensor_copy(csA[:], ctxA[:])
                    csB = cxp.tile([65, 512], F32, name="csB", tag="cs")
                    nc.scalar.copy(csB[:], ctxB[:])
                    for pi in range(4):
                        pieces.append(piece(csA, sc, hA, pi))
                        pieces.append(piece(csB, sc, hB, pi))

        # drain
        while work:
            work.popleft()[1]()
        piece_tags[0] = ("x0", "x1", "grp", "grp")
        while pieces:
            pieces.popleft()()

    nc.compile()
    return nc


def kernel(Q, Wq, bq, Wk, bk, Wv, bv):
    global _cached_nc, last_result
    Q = np.asarray(Q, dtype=np.float32)
    Wq, Wk, Wv = (np.asarray(w, dtype=np.float32) for w in (Wq, Wk, Wv))
    bq, bk, bv = (np.asarray(b, dtype=np.float32) for b in (bq, bk, bv))
    B = Q.shape[0]
    assert Q.shape == (B, S, D) and B * 4 == N_CORES

    if _cached_nc is None:
        _cached_nc = _build()
    nc = _cached_nc

    # host-side shard prep (bf16 inputs: full PE rate, half the DMA traffic)
    bf16 = ml_dtypes.bfloat16
    qts = [np.ascontiguousarray(Q[b].T).astype(bf16) for b in range(B)]
    wqs = [np.ascontiguousarray(Wq[g * J : (g + 1) * J, :].T).astype(bf16) for g in range(4)]
    wks = [np.ascontiguousarray(Wk[g * J : (g + 1) * J, :].T).astype(bf16) for g in range(4)]
    wvs = [np.ascontiguousarray(Wv[g * J : (g + 1) * J, :].T).astype(bf16) for g in range(4)]

    in_maps = []
    for c in range(N_CORES):
        b, g = c // 4, c % 4
        jsl = slice(g * J, (g + 1) * J)
        in_maps.append(
            {
                "qt": qts[b],
                "wq": wqs[g],
                "wk": wks[g],
                "wv": wvs[g],
                "bq": np.ascontiguousarray(bq[jsl]),
                "bk": np.ascontiguousarray(bk[jsl]),
                "bv": np.ascontiguousarray(bv[jsl]),
            }
        )

    last_result = run_bass_kernel_spmd(nc, in_maps, list(range(N_CORES)))

    full = np.empty((B, S, D), dtype=np.float32)
    for c in range(N_CORES):
        b, g = c // 4, c % 4
        full[b, :, g * J : (g + 1) * J] = last_result.results[c]["out"]
    return full


# revision 58
# speedup vs baseline: 1.0184x; 1.0018x over previous
"""Multi-head self-attention Trainium2 kernel (8 NeuronCores, SPMD).

Problem: B=2, S=2048, D=1024, H=16, Dk=64; torch-style Linear projections
(x @ W.T + b), custom softmax: p = exp(scores/8), attn = p / (sum(p) + 1e-8).

Sharding: 32 (batch, head) pairs over 8 cores -> core c handles batch c//4,
heads [4*(c%4), 4*(c%4)+4). Each core projects only its 256 features of
q/k/v; attention is embarrassingly parallel over (b, h).

Per-core kernel: matmuls in fp32r (fp32 with 11 mantissa bits, full PE rate
at moving-dim >= 256). The PE is the bottleneck engine (~165 us of matmul
rows), so the whole kernel is ONE continuous PE stream with no phase
barriers:
  - prologue: kT[0]/v for s-chunks 0-1 and qT[0] chunk 0 (PE ramps while
    the remaining qt chunks DMA in)
  - 128-step attention pipeline over 8 (pair, s-chunk) blocks; the
    remaining projections (kT[0] c2-3, v c2-3, qT[0] c1-3, all of pair 1)
    run as deadline-paced filler between steps, keeping the PE dense so
    the HAM clock gate never rethrottles it
  - exp is split across engines: most t-steps on ACT (table exp, bf16 out),
    5/16 on DVE via a one-instruction Schraudolph in the bf16 bit domain
    (bits16 = trunc(scores*A + B) -> bitcast bf16; ~3% pointwise, mostly
    cancelled by the sum-normalization; ~8e-3 end-to-end vs 2e-2 budget)
  - v and p are bf16 (matmul rate is identical, halves the exp-side SBUF
    and removes the fp32r-producer constraint); scores/q/k stay fp32r --
    fp8 q/k was measured at 2.1e-2 end-to-end and rejected
  - finalize per block: ctx [65,512] PSUM -> SBUF copy (ACT), reciprocal
    in-place on the denominator row (DVE), then per-128-col PE transpose
    and one DVE scalar_tensor_tensor out = ctx*(1/den) + bv reading the
    reciprocal straight from PSUM col 64

Output per core: [2048, 256] fp32 -> host concatenates features per batch.
"""

import sys

sys.path.insert(0, "/opt/trn_rl_repo")

from collections import deque
from contextlib import ExitStack

import ml_dtypes
import numpy as np

import concourse.bass as bass
import concourse.tile as tile
from concourse import bacc, mybir
from concourse.bass_utils import run_bass_kernel_spmd
from concourse.masks import make_identity

F32 = mybir.dt.float32
F32R = mybir.dt.float32r
BF16 = mybir.dt.bfloat16
U16 = mybir.dt.uint16

# Schraudolph exp on DVE, in the bf16 bit domain (the BIR verifier forbids
# int-typed producers feeding fp32r matmuls; bf16 has no such rule):
# exp(g/8) ~= bitcast_bf16(trunc(g*A + B)) via the f32->u16 convert-on-write.
# A = 2^7*log2(e)/8; the down-bias C=5.1 minimizes max pointwise rel err.
SCH_A = float(np.float32((1 << 7) / (8.0 * np.log(2.0))))
SCH_B = float(np.float32((127 << 7) - 5.1))
# t-steps per 16-step block whose exp runs on DVE (rest on ACT)
SCH_STEPS = frozenset({2, 5, 8, 11, 14})

S = 2048  # sequence length
D = 1024  # d_model
J = 256  # features per core (4 heads x 64)
NKT = 8  # k-tiles of the d_model contraction
NSC = 4  # s-chunks of 512
NTT = 16  # t-tiles of 128
N_CORES = 8

_cached_nc = None
last_result = None  # BassKernelResults of the most recent run (for test.py)


def _round_fp32r(x: np.ndarray) -> np.ndarray:
    """Round fp32 to fp32r (keep 11 mantissa bits, round to nearest even)."""
    u = np.ascontiguousarray(x, dtype=np.float32).view(np.uint32)
    r = (u.astype(np.uint64) + 0x7FF + ((u >> 12) & 1)) & 0xFFFFF000
    return r.astype(np.uint32).view(np.float32)


def _build():
    nc = bacc.Bacc(None, target_bir_lowering=False)

    qt = nc.dram_tensor("qt", [D, S], BF16, kind="ExternalInput")
    wq = nc.dram_tensor("wq", [D, J], BF16, kind="ExternalInput")
    wk = nc.dram_tensor("wk", [D, J], BF16, kind="ExternalInput")
    wv = nc.dram_tensor("wv", [D, J], BF16, kind="ExternalInput")
    bq = nc.dram_tensor("bq", [J], F32, kind="ExternalInput")
    bk = nc.dram_tensor("bk", [J], F32, kind="ExternalInput")
    bv = nc.dram_tensor("bv", [J], F32, kind="ExternalInput")
    out = nc.dram_tensor("out", [S, J], F32, kind="ExternalOutput")

    with tile.TileContext(nc) as tc, ExitStack() as ctx:
        wts = ctx.enter_context(tc.tile_pool(name="wts", bufs=1))
        qkp = ctx.enter_context(tc.tile_pool(name="qkp", bufs=1))
        vxp = ctx.enter_context(tc.tile_pool(name="vxp", bufs=1))
        bp = ctx.enter_context(tc.tile_pool(name="bp", bufs=1))
        cxp = ctx.enter_context(tc.tile_pool(name="cxp", bufs=12))
        pTp = ctx.enter_context(tc.tile_pool(name="pTp", bufs=6))
        outp = ctx.enter_context(tc.tile_pool(name="outp", bufs=1))
        qtcp = ctx.enter_context(tc.tile_pool(name="qtc", bufs=1))
        rp = ctx.enter_context(tc.tile_pool(name="rp", bufs=8))
        aps = ctx.enter_context(tc.tile_pool(name="aps", bufs=1, space="PSUM"))
        p1b = ctx.enter_context(tc.tile_pool(name="p1b", bufs=1, space="PSUM"))

        # Weights: 8 k-tiles each of [128, 256], k-major and split across the
        # HWDGE (sync/scalar) / SWDGE (gpsimd) queues, interleaved with the
        # first s-chunk of QT so the first projection matmuls start early
        wq_t = [
            wts.tile([128, J], BF16, name=f"wq{k}", tag=f"wq{k}") for k in range(NKT)
        ]
        wk_t = [
            wts.tile([128, J], BF16, name=f"wk{k}", tag=f"wk{k}") for k in range(NKT)
        ]
        wv_t = [
            wts.tile([128, J], BF16, name=f"wv{k}", tag=f"wv{k}") for k in range(NKT)
        ]
        qtc = [
            qtcp.tile([128, NKT, 512], BF16, name=f"qtc{c}", tag=f"qtc{c}")
            for c in range(NSC)
        ]
        # qtc chunk k-tiles alternate sync/vector HWDGE queues so the first
        # k-tiles land fast; wk streams ahead of wq on scalar (the k chain
        # runs first); wv + biases go via gpsimd SWDGE
        # ident is built FIRST on gpsimd (before its DMA-issue backlog) so
        # the PE warm-up transposes can start at ~7us
        ident = bp.tile([128, 128], F32, name="ident")
        make_identity(nc, ident[:])

        # ALL qtc issues on sync: the scalar sequencer must stay shallow --
        # its DMA-issue backlog (0.6us each, in-order with engine ops) would
        # otherwise gate the first exp
        for k in range(NKT):
            ksl = slice(k * 128, (k + 1) * 128)
            nc.sync.dma_start(qtc[0][:, k, :], qt[ksl, 0:512])
            nc.scalar.dma_start(wk_t[k][:], wk[ksl, :])
        for k in range(NKT):
            ksl = slice(k * 128, (k + 1) * 128)
            nc.scalar.dma_start(wq_t[k][:], wq[ksl, :])
            nc.gpsimd.dma_start(wv_t[k][:], wv[ksl, :])
        # chunk 1 is needed soonest (attention t=4..7 plus v s-tiles 4-7):
        # split it across both queues; chunks 2-3 follow on sync
        for k in range(NKT):
            eng = nc.sync if k % 2 == 0 else nc.scalar
            eng.dma_start(qtc[1][:, k, :], qt[k * 128 : (k + 1) * 128, 512:1024])
        for c in range(2, NSC):
            s0 = c * 512
            for k in range(NKT):
                nc.sync.dma_start(
                    qtc[c][:, k, :], qt[k * 128 : (k + 1) * 128, s0 : s0 + 512]
                )

        # Biases: bq/bk as per-partition scalars [128, 2]; bv broadcast [128, 256]
        bq_t = bp.tile([128, 2], F32, name="bqt")
        nc.gpsimd.dma_start(bq_t[:], bq.rearrange("(m p) -> p m", p=128))
        bk_t = bp.tile([128, 2], F32, name="bkt")
        nc.gpsimd.dma_start(bk_t[:], bk.rearrange("(m p) -> p m", p=128))
        bv_t = bp.tile([128, J], F32, name="bvt")
        bvap = bv[:]
        bv_bcast = bass.AP(
            tensor=bvap.tensor, offset=bvap.offset, ap=[[0, 128], [1, J]]
        )
        nc.gpsimd.dma_start(bv_t[:], bv_bcast)

        scratch = bp.tile([128, 1], F32, name="scratch")

        # Persistent projected tensors
        qT = [qkp.tile([128, S], BF16, name=f"qT{m}", tag=f"qT{m}") for m in range(2)]
        kT = [qkp.tile([128, S], BF16, name=f"kT{m}", tag=f"kT{m}") for m in range(2)]
        v_ext = []
        for t in range(NTT):
            vt = vxp.tile([128, 4, 65], BF16, name=f"vx{t}", tag=f"vx{t}")
            # DVE memsets (it is idle during the prologue; gpsimd's queue is
            # busy streaming wv and would delay the first v epilogues)
            nc.vector.memset(vt[:], 1.0)  # ones col [:, h, 64] survives
            v_ext.append(vt)
        out_tiles = [
            outp.tile([128, J], F32, name=f"ot{b}", tag=f"ot{b}") for b in range(16)
        ]

        # pre-load the ACT exp table (it only needs bq_t) so the first
        # attention exp doesn't pay the ~2.7us table-load stall
        nc.scalar.activation(
            scratch[:], bq_t[:, 0:1], mybir.ActivationFunctionType.Exp, scale=0.0
        )

        # ---------- projection instruction chains ----------
        xrot = [0]

        def xtile(shape=(128, 512)):
            t_ = p1b.tile(list(shape), F32, name="px", tag=f"x{xrot[0] % 2}")
            xrot[0] += 1
            return t_

        def emit_qk_chain(pair, c, which, korder=None):
            """8 matmuls + bias epilogue for qT/kT[pair][:, c*512:(c+1)*512]."""
            st = {}
            w_t = wq_t if which == "q" else wk_t
            dst = (qT if which == "q" else kT)[pair]
            b_t = bq_t if which == "q" else bk_t
            ks = list(range(NKT)) if korder is None else korder

            def mm(k):
                def f():
                    if k == ks[0]:
                        st["t"] = xtile()
                    nc.tensor.matmul(
                        st["t"][:],
                        w_t[k][:, pair * 128 : (pair + 1) * 128],
                        qtc[c][:, k, :],
                        start=(k == ks[0]),
                        stop=(k == ks[-1]),
                    )
                return f

            def epi():
                s0 = c * 512
                nc.vector.tensor_scalar_add(
                    dst[:, s0 : s0 + 512], st.pop("t")[:], b_t[:, pair : pair + 1]
                )

            return [mm(k) for k in ks] + [epi]

        def emit_pv_chain(c, half):
            """v for s-tiles (4c+2*half, 4c+2*half+1): 16 matmuls packed into
            one [128,512] PSUM bank (two 256-col accumulation groups; the
            second group starts on still-pending-zero bytes), + 2 epilogues."""
            st = {}
            thunks = []

            def mm(k, sub):
                def f():
                    if k == 0 and sub == 0:
                        st["t"] = xtile()
                    i = 2 * half + sub
                    nc.tensor.matmul(
                        st["t"][:, sub * 256 : (sub + 1) * 256],
                        qtc[c][:, k, i * 128 : (i + 1) * 128],
                        wv_t[k][:],
                        start=(k == 0 and sub == 0),
                        stop=(k == NKT - 1 and sub == 1),
                        skip_group_check=True,
                    )
                return f

            for sub in range(2):
                for k in range(NKT):
                    thunks.append(mm(k, sub))

            def epi(sub):
                def f():
                    i = 2 * half + sub
                    src = st["t"][:, sub * 256 : (sub + 1) * 256]
                    nc.vector.tensor_copy(
                        v_ext[c * 4 + i][:, :, 0:64],
                        src.rearrange("p (h d) -> p h d", h=4),
                    )
                    if sub == 1:
                        st.pop("t")
                return f

            thunks += [epi(0), epi(1)]
            return thunks

        # ---------- prologue: minimum prefix (kT[0]/qT[0] chunk 0) ----------
        for th in emit_qk_chain(0, 0, "k") + emit_qk_chain(0, 0, "q"):
            th()

        # ---------- filler: remaining projections, deadline-paced ----------
        chains = [
            (emit_pv_chain(0, 0), 0),
            (emit_pv_chain(0, 1), 1),
            (emit_qk_chain(0, 1, "k"), 3),
            (emit_pv_chain(1, 0), 4),
            (emit_pv_chain(1, 1), 5),
            (emit_qk_chain(0, 2, "k"), 7),
            (emit_pv_chain(2, 0), 8),
            (emit_pv_chain(2, 1), 9),
            (emit_qk_chain(0, 3, "k"), 11),
            (emit_pv_chain(3, 0), 12),
            (emit_pv_chain(3, 1), 13),
            (emit_qk_chain(0, 1, "q"), 15),
            (emit_qk_chain(0, 2, "q"), 30),
            (emit_qk_chain(0, 3, "q"), 42),
            (emit_qk_chain(1, 0, "k"), 48),
            (emit_qk_chain(1, 0, "q"), 54),
            (emit_qk_chain(1, 1, "k"), 60),
            (emit_qk_chain(1, 1, "q"), 64),
            (emit_qk_chain(1, 2, "k"), 68),
            (emit_qk_chain(1, 2, "q"), 71),
            (emit_qk_chain(1, 3, "k"), 74),
            (emit_qk_chain(1, 3, "q"), 77),
        ]
        work = deque()
        prev_dl = 0
        for thunks, dl in chains:
            n = len(thunks)
            for j, th in enumerate(thunks):
                work.append((prev_dl + (dl - prev_dl) * (j + 1) / n, th))
            prev_dl = dl

        # ---------- attention: one continuous 128-step pipeline ----------
        pieces = deque()
        done_cnt = {}
        piece_tags = [("x0", "x1")]

        def piece(cs_tile, sc, h, pi):
            def f():
                tags = piece_tags[0]
                tag = tags[xrot[0] % len(tags)]
                xrot[0] += 1
                if tag.startswith("x"):
                    tp = p1b.tile([128, 65], F32, name="tp", tag=tag)
                else:
                    tp = aps.tile([128, 65], F32, name="tp", tag=tag, bufs=2)
                nc.tensor.transpose(
                    tp[:],
                    cs_tile[0:65, pi * 128 : (pi + 1) * 128],
                    ident[0:65, 0:65],
                )
                blk = sc * 4 + pi
                r = rp.tile([128, 1], F32, name="r", tag="r")
                nc.vector.reciprocal(r[:], tp[:, 64:65])
                nc.vector.scalar_tensor_tensor(
                    out=out_tiles[blk][:, h * 64 : (h + 1) * 64],
                    in0=tp[:, 0:64],
                    scalar=r[:],
                    in1=bv_t[:, h * 64 : (h + 1) * 64],
                    op0=mybir.AluOpType.mult,
                    op1=mybir.AluOpType.add,
                )
                key = (blk, h // 2)
                done_cnt[key] = done_cnt.get(key, 0) + 1
                if done_cnt[key] == 2:
                    jsl = slice((h // 2) * 128, (h // 2) * 128 + 128)
                    eng = nc.sync if blk % 2 == 0 else nc.scalar
                    eng.dma_start(
                        out[blk * 128 : (blk + 1) * 128, jsl],
                        out_tiles[blk][:, jsl],
                    )
            return f

        def fill_slot(i):
            popped = False
            while work and work[0][0] <= i:
                work.popleft()[1]()
                popped = True
            if pieces and not work and not popped:
                pieces.popleft()()
                if len(pieces) > 16 and pieces:
                    pieces.popleft()()

        blocks = [(p, sc) for p in range(2) for sc in range(NSC)]
        NB = len(blocks)
        LAG = 2  # AV issued 2 steps behind its exp: the sem has long fired,
        #          so the in-order PE never stalls at the AV queue head
        ctx_ps = {}
        pts = {}
        for i in range(NB * NTT + LAG):
            if i < NB * NTT:
                b, t = divmod(i, NTT)
                pair, sc = blocks[b]
                s0 = sc * 512
                qTt, kTt = qT[pair], kT[pair]
                if t == 0:
                    ctxA = aps.tile([65, 512], F32, name="ctxA", tag="ctx", bufs=2)
                    ctxB = aps.tile([65, 512], F32, name="ctxB", tag="ctx", bufs=2)
                    ctx_ps[b] = (ctxA, ctxB)
                tsl = slice(t * 128, (t + 1) * 128)
                # both heads' scoresT share one 2-bank tile so a single
                # exp instruction covers both
                g = aps.tile([128, 1024], F32, name="g", tag="grp", bufs=2)
                nc.tensor.matmul(
                    g[:, 0:512],
                    kTt[0:64, tsl],
                    qTt[0:64, s0 : s0 + 512],
                    start=True,
                    stop=True,
                    tile_position=(0, 0),
                )
                nc.tensor.matmul(
                    g[:, 512:1024],
                    kTt[64:128, tsl],
                    qTt[64:128, s0 : s0 + 512],
                    start=True,
                    stop=True,
                    tile_position=(64, 0),
                )
                pT_ = pTp.tile([128, 1024], BF16, name="pT_", tag="pT")
                if t in SCH_STEPS:
                    nc.vector.tensor_scalar(
                        out=pT_[:].bitcast(U16),
                        in0=g[:],
                        scalar1=SCH_A,
                        scalar2=SCH_B,
                        op0=mybir.AluOpType.mult,
                        op1=mybir.AluOpType.add,
                    )
                else:
                    nc.scalar.activation(
                        pT_[:], g[:],
                        mybir.ActivationFunctionType.Exp, scale=0.125,
                    )
                pts[i] = pT_
            # filler between exp-issue and the AV that consumes the previous
            # exp: the PE chews projections/transposes instead of stalling at
            # the queue head while ACT finishes
            fill_slot(i)
            if i >= LAG:
                b, t = divmod(i - LAG, NTT)
                pair, sc = blocks[b]
                hA, hB = 2 * pair, 2 * pair + 1
                ctxA, ctxB = ctx_ps[b]
                pT_ = pts.pop(i - LAG)
                st_, sp_ = (t == 0), (t == NTT - 1)
                nc.tensor.matmul(
                    ctxA[:], v_ext[t][:, hA, :], pT_[:, 0:512],
                    start=st_, stop=sp_,
                )
                nc.tensor.matmul(
                    ctxB[:], v_ext[t][:, hB, :], pT_[:, 512:1024],
                    start=st_, stop=sp_,
                )
                if t == NTT - 1:
                    del ctx_ps[b]
                    csA = cxp.tile([65, 512], F32, name="csA", tag="cs")
                    nc.vector.tensor_copy(csA[:], ctxA[:])
                    csB = cxp.tile([65, 512], F32, name="csB", tag="cs")
                    nc.scalar.copy(csB[:], ctxB[:])
                    for pi in range(4):
                        pieces.append(piece(csA, sc, hA, pi))
                        pieces.append(piece(csB, sc, hB, pi))

        # drain
        while work:
            work.popleft()[1]()
        piece_tags[0] = ("x0", "x1", "grp", "grp")
        while pieces:
            pieces.popleft()()

    nc.compile()
    return nc


def kernel(Q, Wq, bq, Wk, bk, Wv, bv):
    global _cached_nc, last_result
    Q = np.asarray(Q, dtype=np.float32)
    Wq, Wk, Wv = (np.asarray(w, dtype=np.float32) for w in (Wq, Wk, Wv))
    bq, bk, bv = (np.asarray(b, dtype=np.float32) for b in (bq, bk, bv))
    B = Q.shape[0]
    assert Q.shape == (B, S, D) and B * 4 == N_CORES

    if _cached_nc is None:
        _cached_nc = _build()
    nc = _cached_nc

    # host-side shard prep (bf16 inputs: full PE rate, half the DMA traffic)
    bf16 = ml_dtypes.bfloat16
    qts = [np.ascontiguousarray(Q[b].T).astype(bf16) for b in range(B)]
    wqs = [np.ascontiguousarray(Wq[g * J : (g + 1) * J, :].T).astype(bf16) for g in range(4)]
    wks = [np.ascontiguousarray(Wk[g * J : (g + 1) * J, :].T).astype(bf16) for g in range(4)]
    wvs = [np.ascontiguousarray(Wv[g * J : (g + 1) * J, :].T).astype(bf16) for g in range(4)]

    in_maps = []
    for c in range(N_CORES):
        b, g = c // 4, c % 4
        jsl = slice(g * J, (g + 1) * J)
        in_maps.append(
            {
                "qt": qts[b],
                "wq": wqs[g],
                "wk": wks[g],
                "wv": wvs[g],
                "bq": np.ascontiguousarray(bq[jsl]),
                "bk": np.ascontiguousarray(bk[jsl]),
                "bv": np.ascontiguousarray(bv[jsl]),
            }
        )

    last_result = run_bass_kernel_spmd(nc, in_maps, list(range(N_CORES)))

    full = np.empty((B, S, D), dtype=np.float32)
    for c in range(N_CORES):
        b, g = c // 4, c % 4
        full[b, :, g * J : (g + 1) * J] = last_result.results[c]["out"]
    return full


# revision 59
# speedup vs baseline: 1.0263x; 1.0077x over previous
"""Multi-head self-attention Trainium2 kernel (8 NeuronCores, SPMD).

Problem: B=2, S=2048, D=1024, H=16, Dk=64; torch-style Linear projections
(x @ W.T + b), custom softmax: p = exp(scores/8), attn = p / (sum(p) + 1e-8).

Sharding: 32 (batch, head) pairs over 8 cores -> core c handles batch c//4,
heads [4*(c%4), 4*(c%4)+4). Each core projects only its 256 features of
q/k/v; attention is embarrassingly parallel over (b, h).

All matmul operands are bf16 (same PE rate as fp32r at these shapes, but
half the DMA traffic and LDWEIGHTS time; fp32 accumulation in PSUM keeps
the contractions exact). fp8 q/k via DoubleRow was measured at 2.1e-2
end-to-end and rejected. The PE is the bottleneck engine (~155-165 us of
matmul rows), so the whole kernel is ONE continuous PE instruction stream
with no phase barriers:
  - prologue: only kT[0]/qT[0] chunk 0 (17 matmuls); everything else is
    deadline-paced filler inside the attention loop
  - 128-step attention pipeline over 8 (pair, s-chunk) blocks. Per step:
    2 score matmuls (tile_position row-groups run concurrently in the PE
    array), one exp covering both heads, 2 AV matmuls issued with a 2-step
    LAG so their exp semaphores have long fired when they reach the head
    of the in-order PE queue. Filler pops BETWEEN the exp and the AVs so
    the PE chews projections instead of stalling.
  - filler: remaining projections (kT[0] c1-3, all of v, qT[0] c1-3, all
    of pair 1) with per-unit deadlines derived from first-use steps; the
    x0/x1 PSUM banks rotate between accumulation chains. v s-tile pairs
    pack two 256-col accumulation groups into one 2KB PSUM bank (the
    second group starts on still-pending-zero bytes).
  - exp split across engines: 11/16 t-steps on ACT (table exp, bf16 out),
    5/16 on DVE via a one-instruction Schraudolph in the bf16 bit domain
    (bits16 = trunc(scores*A + B) via f32->u16 convert-on-write, bitcast
    to bf16; ~3% pointwise, mostly cancelled by the sum-normalization;
    8.4e-3 end-to-end vs the 2e-2 budget)
  - DMA layout: the scalar sequencer's issue backlog would gate the first
    exp (DIRECT2D issues are ~0.6us each, in-order with engine ops), so
    qt rides sync (chunk 1 split across both queues), weights ride
    scalar, wv/biases ride gpsimd SWDGE
  - finalize per block: ctx [65,512] PSUM -> SBUF copies (split DVE/ACT),
    per-128-col PE transpose, [128,1] DVE reciprocal of the transposed
    denominator column, one DVE scalar_tensor_tensor out = ctx/den + bv;
    each out tile's column-halves DMA out as soon as their two pieces
    land (the pair-0 half leaves ~40us before the pair-1 half)

Output per core: [2048, 256] fp32 -> host concatenates features per batch.
"""

import sys

sys.path.insert(0, "/opt/trn_rl_repo")

from collections import deque
from contextlib import ExitStack

import ml_dtypes
import numpy as np

import concourse.bass as bass
import concourse.tile as tile
from concourse import bacc, mybir
from concourse.bass_utils import run_bass_kernel_spmd
from concourse.masks import make_identity

F32 = mybir.dt.float32
F32R = mybir.dt.float32r
BF16 = mybir.dt.bfloat16
U16 = mybir.dt.uint16

# Schraudolph exp on DVE, in the bf16 bit domain (the BIR verifier forbids
# int-typed producers feeding fp32r matmuls; bf16 has no such rule):
# exp(g/8) ~= bitcast_bf16(trunc(g*A + B)) via the f32->u16 convert-on-write.
# A = 2^7*log2(e)/8; the down-bias C=5.1 minimizes max pointwise rel err.
SCH_A = float(np.float32((1 << 7) / (8.0 * np.log(2.0))))
SCH_B = float(np.float32((127 << 7) - 5.1))
# t-steps per 16-step block whose exp runs on DVE (rest on ACT)
SCH_STEPS = frozenset({2, 5, 8, 11, 14})

S = 2048  # sequence length
D = 1024  # d_model
J = 256  # features per core (4 heads x 64)
NKT = 8  # k-tiles of the d_model contraction
NSC = 4  # s-chunks of 512
NTT = 16  # t-tiles of 128
N_CORES = 8

_cached_nc = None
last_result = None  # BassKernelResults of the most recent run (for test.py)


def _round_fp32r(x: np.ndarray) -> np.ndarray:
    """Round fp32 to fp32r (keep 11 mantissa bits, round to nearest even)."""
    u = np.ascontiguousarray(x, dtype=np.float32).view(np.uint32)
    r = (u.astype(np.uint64) + 0x7FF + ((u >> 12) & 1)) & 0xFFFFF000
    return r.astype(np.uint32).view(np.float32)


def _build():
    nc = bacc.Bacc(None, target_bir_lowering=False)

    qt = nc.dram_tensor("qt", [D, S], BF16, kind="ExternalInput")
    wq = nc.dram_tensor("wq", [D, J], BF16, kind="ExternalInput")
    wk = nc.dram_tensor("wk", [D, J], BF16, kind="ExternalInput")
    wv = nc.dram_tensor("wv", [D, J], BF16, kind="ExternalInput")
    bq = nc.dram_tensor("bq", [J], F32, kind="ExternalInput")
    bk = nc.dram_tensor("bk", [J], F32, kind="ExternalInput")
    bv = nc.dram_tensor("bv", [J], F32, kind="ExternalInput")
    out = nc.dram_tensor("out", [S, J], F32, kind="ExternalOutput")

    with tile.TileContext(nc) as tc, ExitStack() as ctx:
        wts = ctx.enter_context(tc.tile_pool(name="wts", bufs=1))
        qkp = ctx.enter_context(tc.tile_pool(name="qkp", bufs=1))
        vxp = ctx.enter_context(tc.tile_pool(name="vxp", bufs=1))
        bp = ctx.enter_context(tc.tile_pool(name="bp", bufs=1))
        cxp = ctx.enter_context(tc.tile_pool(name="cxp", bufs=12))
        pTp = ctx.enter_context(tc.tile_pool(name="pTp", bufs=6))
        outp = ctx.enter_context(tc.tile_pool(name="outp", bufs=1))
        qtcp = ctx.enter_context(tc.tile_pool(name="qtc", bufs=1))
        rp = ctx.enter_context(tc.tile_pool(name="rp", bufs=8))
        aps = ctx.enter_context(tc.tile_pool(name="aps", bufs=1, space="PSUM"))
        p1b = ctx.enter_context(tc.tile_pool(name="p1b", bufs=1, space="PSUM"))

        # Weights: 8 k-tiles each of [128, 256], k-major and split across the
        # HWDGE (sync/scalar) / SWDGE (gpsimd) queues, interleaved with the
        # first s-chunk of QT so the first projection matmuls start early
        wq_t = [
            wts.tile([128, J], BF16, name=f"wq{k}", tag=f"wq{k}") for k in range(NKT)
        ]
        wk_t = [
            wts.tile([128, J], BF16, name=f"wk{k}", tag=f"wk{k}") for k in range(NKT)
        ]
        wv_t = [
            wts.tile([128, J], BF16, name=f"wv{k}", tag=f"wv{k}") for k in range(NKT)
        ]
        qtc = [
            qtcp.tile([128, NKT, 512], BF16, name=f"qtc{c}", tag=f"qtc{c}")
            for c in range(NSC)
        ]
        # qtc chunk k-tiles alternate sync/vector HWDGE queues so the first
        # k-tiles land fast; wk streams ahead of wq on scalar (the k chain
        # runs first); wv + biases go via gpsimd SWDGE
        # ident is built FIRST on gpsimd (before its DMA-issue backlog) so
        # the PE warm-up transposes can start at ~7us
        ident = bp.tile([128, 128], F32, name="ident")
        make_identity(nc, ident[:])

        # ALL qtc issues on sync: the scalar sequencer must stay shallow --
        # its DMA-issue backlog (0.6us each, in-order with engine ops) would
        # otherwise gate the first exp
        for k in range(NKT):
            ksl = slice(k * 128, (k + 1) * 128)
            nc.sync.dma_start(qtc[0][:, k, :], qt[ksl, 0:512])
            nc.scalar.dma_start(wk_t[k][:], wk[ksl, :])
        for k in range(NKT):
            ksl = slice(k * 128, (k + 1) * 128)
            nc.scalar.dma_start(wq_t[k][:], wq[ksl, :])
            nc.gpsimd.dma_start(wv_t[k][:], wv[ksl, :])
        # chunk 1 is needed soonest (attention t=4..7 plus v s-tiles 4-7):
        # split it across both queues; chunks 2-3 follow on sync
        for k in range(NKT):
            eng = nc.sync if k % 2 == 0 else nc.scalar
            eng.dma_start(qtc[1][:, k, :], qt[k * 128 : (k + 1) * 128, 512:1024])
        for c in range(2, NSC):
            s0 = c * 512
            for k in range(NKT):
                nc.sync.dma_start(
                    qtc[c][:, k, :], qt[k * 128 : (k + 1) * 128, s0 : s0 + 512]
                )

        # Biases: bq/bk as per-partition scalars [128, 2]; bv broadcast [128, 256]
        bq_t = bp.tile([128, 2], F32, name="bqt")
        nc.gpsimd.dma_start(bq_t[:], bq.rearrange("(m p) -> p m", p=128))
        bk_t = bp.tile([128, 2], F32, name="bkt")
        nc.gpsimd.dma_start(bk_t[:], bk.rearrange("(m p) -> p m", p=128))
        bv_t = bp.tile([128, J], F32, name="bvt")
        bvap = bv[:]
        bv_bcast = bass.AP(
            tensor=bvap.tensor, offset=bvap.offset, ap=[[0, 128], [1, J]]
        )
        nc.gpsimd.dma_start(bv_t[:], bv_bcast)

        scratch = bp.tile([128, 1], F32, name="scratch")

        # Persistent projected tensors
        qT = [qkp.tile([128, S], BF16, name=f"qT{m}", tag=f"qT{m}") for m in range(2)]
        kT = [qkp.tile([128, S], BF16, name=f"kT{m}", tag=f"kT{m}") for m in range(2)]
        v_ext = []
        for t in range(NTT):
            vt = vxp.tile([128, 4, 65], BF16, name=f"vx{t}", tag=f"vx{t}")
            # DVE memsets (it is idle during the prologue; gpsimd's queue is
            # busy streaming wv and would delay the first v epilogues)
            nc.vector.memset(vt[:], 1.0)  # ones col [:, h, 64] survives
            v_ext.append(vt)
        out_tiles = [
            outp.tile([128, J], F32, name=f"ot{b}", tag=f"ot{b}") for b in range(16)
        ]

        # pre-load the ACT exp table (it only needs bq_t) so the first
        # attention exp doesn't pay the ~2.7us table-load stall
        nc.scalar.activation(
            scratch[:], bq_t[:, 0:1], mybir.ActivationFunctionType.Exp, scale=0.0
        )

        # ---------- projection instruction chains ----------
        xrot = [0]

        def xtile(shape=(128, 512)):
            t_ = p1b.tile(list(shape), F32, name="px", tag=f"x{xrot[0] % 2}")
            xrot[0] += 1
            return t_

        def emit_qk_chain(pair, c, which, korder=None):
            """8 matmuls + bias epilogue for qT/kT[pair][:, c*512:(c+1)*512]."""
            st = {}
            w_t = wq_t if which == "q" else wk_t
            dst = (qT if which == "q" else kT)[pair]
            b_t = bq_t if which == "q" else bk_t
            ks = list(range(NKT)) if korder is None else korder

            def mm(k):
                def f():
                    if k == ks[0]:
                        st["t"] = xtile()
                    nc.tensor.matmul(
                        st["t"][:],
                        w_t[k][:, pair * 128 : (pair + 1) * 128],
                        qtc[c][:, k, :],
                        start=(k == ks[0]),
                        stop=(k == ks[-1]),
                    )
                return f

            def epi():
                s0 = c * 512
                nc.vector.tensor_scalar_add(
                    dst[:, s0 : s0 + 512], st.pop("t")[:], b_t[:, pair : pair + 1]
                )

            return [mm(k) for k in ks] + [epi]

        def emit_pv_chain(c, half):
            """v for s-tiles (4c+2*half, 4c+2*half+1): 16 matmuls packed into
            one [128,512] PSUM bank (two 256-col accumulation groups; the
            second group starts on still-pending-zero bytes), + 2 epilogues."""
            st = {}
            thunks = []

            def mm(k, sub):
                def f():
                    if k == 0 and sub == 0:
                        st["t"] = xtile()
                    i = 2 * half + sub
                    nc.tensor.matmul(
                        st["t"][:, sub * 256 : (sub + 1) * 256],
                        qtc[c][:, k, i * 128 : (i + 1) * 128],
                        wv_t[k][:],
                        start=(k == 0 and sub == 0),
                        stop=(k == NKT - 1 and sub == 1),
                        skip_group_check=True,
                    )
                return f

            for sub in range(2):
                for k in range(NKT):
                    thunks.append(mm(k, sub))

            def epi(sub):
                def f():
                    i = 2 * half + sub
                    src = st["t"][:, sub * 256 : (sub + 1) * 256]
                    nc.vector.tensor_copy(
                        v_ext[c * 4 + i][:, :, 0:64],
                        src.rearrange("p (h d) -> p h d", h=4),
                    )
                    if sub == 1:
                        st.pop("t")
                return f

            thunks += [epi(0), epi(1)]
            return thunks

        # ---------- prologue: minimum prefix (kT[0]/qT[0] chunk 0) ----------
        for th in emit_qk_chain(0, 0, "k") + emit_qk_chain(0, 0, "q"):
            th()

        # ---------- filler: remaining projections, deadline-paced ----------
        chains = [
            (emit_pv_chain(0, 0), 0),
            (emit_pv_chain(0, 1), 1),
            (emit_qk_chain(0, 1, "k"), 3),
            (emit_pv_chain(1, 0), 4),
            (emit_pv_chain(1, 1), 5),
            (emit_qk_chain(0, 2, "k"), 7),
            (emit_pv_chain(2, 0), 8),
            (emit_pv_chain(2, 1), 9),
            (emit_qk_chain(0, 3, "k"), 11),
            (emit_pv_chain(3, 0), 12),
            (emit_pv_chain(3, 1), 13),
            (emit_qk_chain(0, 1, "q"), 15),
            (emit_qk_chain(0, 2, "q"), 30),
            (emit_qk_chain(0, 3, "q"), 42),
            (emit_qk_chain(1, 0, "k"), 48),
            (emit_qk_chain(1, 0, "q"), 54),
            (emit_qk_chain(1, 1, "k"), 60),
            (emit_qk_chain(1, 1, "q"), 64),
            (emit_qk_chain(1, 2, "k"), 68),
            (emit_qk_chain(1, 2, "q"), 71),
            (emit_qk_chain(1, 3, "k"), 74),
            (emit_qk_chain(1, 3, "q"), 77),
        ]
        work = deque()
        prev_dl = 0
        for thunks, dl in chains:
            n = len(thunks)
            for j, th in enumerate(thunks):
                work.append((prev_dl + (dl - prev_dl) * (j + 1) / n, th))
            prev_dl = dl

        # ---------- attention: one continuous 128-step pipeline ----------
        pieces = deque()
        done_cnt = {}
        piece_tags = [("x0", "x1")]

        def piece(cs_tile, sc, h, pi):
            def f():
                tags = piece_tags[0]
                tag = tags[xrot[0] % len(tags)]
                xrot[0] += 1
                if tag.startswith("x"):
                    tp = p1b.tile([128, 65], F32, name="tp", tag=tag)
                else:
                    tp = aps.tile([128, 65], F32, name="tp", tag=tag, bufs=2)
                nc.tensor.transpose(
                    tp[:],
                    cs_tile[0:65, pi * 128 : (pi + 1) * 128],
                    ident[0:65, 0:65],
                )
                blk = sc * 4 + pi
                r = rp.tile([128, 1], F32, name="r", tag="r")
                nc.vector.reciprocal(r[:], tp[:, 64:65])
                nc.vector.scalar_tensor_tensor(
                    out=out_tiles[blk][:, h * 64 : (h + 1) * 64],
                    in0=tp[:, 0:64],
                    scalar=r[:],
                    in1=bv_t[:, h * 64 : (h + 1) * 64],
                    op0=mybir.AluOpType.mult,
                    op1=mybir.AluOpType.add,
                )
                key = (blk, h // 2)
                done_cnt[key] = done_cnt.get(key, 0) + 1
                if done_cnt[key] == 2:
                    jsl = slice((h // 2) * 128, (h // 2) * 128 + 128)
                    eng = nc.sync if blk % 2 == 0 else nc.scalar
                    eng.dma_start(
                        out[blk * 128 : (blk + 1) * 128, jsl],
                        out_tiles[blk][:, jsl],
                    )
            return f

        def fill_slot(i):
            popped = False
            while work and work[0][0] <= i:
                work.popleft()[1]()
                popped = True
            if pieces and not work and not popped:
                pieces.popleft()()
                if len(pieces) > 16 and pieces:
                    pieces.popleft()()

        blocks = [(p, sc) for p in range(2) for sc in range(NSC)]
        NB = len(blocks)
        LAG = 2  # AV issued 2 steps behind its exp: the sem has long fired,
        #          so the in-order PE never stalls at the AV queue head
        ctx_ps = {}
        pts = {}
        for i in range(NB * NTT + LAG):
            if i < NB * NTT:
                b, t = divmod(i, NTT)
                pair, sc = blocks[b]
                s0 = sc * 512
                qTt, kTt = qT[pair], kT[pair]
                if t == 0:
                    ctxA = aps.tile([65, 512], F32, name="ctxA", tag="ctx", bufs=2)
                    ctxB = aps.tile([65, 512], F32, name="ctxB", tag="ctx", bufs=2)
                    ctx_ps[b] = (ctxA, ctxB)
                tsl = slice(t * 128, (t + 1) * 128)
                # both heads' scoresT share one 2-bank tile so a single
                # exp instruction covers both
                g = aps.tile([128, 1024], F32, name="g", tag="grp", bufs=2)
                nc.tensor.matmul(
                    g[:, 0:512],
                    kTt[0:64, tsl],
                    qTt[0:64, s0 : s0 + 512],
                    start=True,
                    stop=True,
                    tile_position=(0, 0),
                )
                nc.tensor.matmul(
                    g[:, 512:1024],
                    kTt[64:128, tsl],
                    qTt[64:128, s0 : s0 + 512],
                    start=True,
                    stop=True,
                    tile_position=(64, 0),
                )
                pT_ = pTp.tile([128, 1024], BF16, name="pT_", tag="pT")
                if t in SCH_STEPS:
                    nc.vector.tensor_scalar(
                        out=pT_[:].bitcast(U16),
                        in0=g[:],
                        scalar1=SCH_A,
                        scalar2=SCH_B,
                        op0=mybir.AluOpType.mult,
                        op1=mybir.AluOpType.add,
                    )
                else:
                    nc.scalar.activation(
                        pT_[:], g[:],
                        mybir.ActivationFunctionType.Exp, scale=0.125,
                    )
                pts[i] = pT_
            # filler between exp-issue and the AV that consumes the previous
            # exp: the PE chews projections/transposes instead of stalling at
            # the queue head while ACT finishes
            fill_slot(i)
            if i >= LAG:
                b, t = divmod(i - LAG, NTT)
                pair, sc = blocks[b]
                hA, hB = 2 * pair, 2 * pair + 1
                ctxA, ctxB = ctx_ps[b]
                pT_ = pts.pop(i - LAG)
                st_, sp_ = (t == 0), (t == NTT - 1)
                nc.tensor.matmul(
                    ctxA[:], v_ext[t][:, hA, :], pT_[:, 0:512],
                    start=st_, stop=sp_,
                )
                nc.tensor.matmul(
                    ctxB[:], v_ext[t][:, hB, :], pT_[:, 512:1024],
                    start=st_, stop=sp_,
                )
                if t == NTT - 1:
                    del ctx_ps[b]
                    csA = cxp.tile([65, 512], F32, name="csA", tag="cs")
                    nc.vector.tensor_copy(csA[:], ctxA[:])
                    csB = cxp.tile([65, 512], F32, name="csB", tag="cs")
                    nc.scalar.copy(csB[:], ctxB[:])
                    for pi in range(4):
                        pieces.append(piece(csA, sc, hA, pi))
                        pieces.append(piece(csB, sc, hB, pi))

        # drain
        while work:
            work.popleft()[1]()
        piece_tags[0] = ("x0", "x1", "grp", "grp")
        while pieces:
            pieces.popleft()()

    nc.compile()
    return nc


def kernel(Q, Wq, bq, Wk, bk, Wv, bv):
    global _cached_nc, last_result
    Q = np.asarray(Q, dtype=np.float32)
    Wq, Wk, Wv = (np.asarray(w, dtype=np.float32) for w in (Wq, Wk, Wv))
    bq, bk, bv = (np.asarray(b, dtype=np.float32) for b in (bq, bk, bv))
    B = Q.shape[0]
    assert Q.shape == (B, S, D) and B * 4 == N_CORES

    if _cached_nc is None:
        _cached_nc = _build()
    nc = _cached_nc

    # host-side shard prep (bf16 inputs: full PE rate, half the DMA traffic)
    bf16 = ml_dtypes.bfloat16
    qts = [np.ascontiguousarray(Q[b].T).astype(bf16) for b in range(B)]
    wqs = [np.ascontiguousarray(Wq[g * J : (g + 1) * J, :].T).astype(bf16) for g in range(4)]
    wks = [np.ascontiguousarray(Wk[g * J : (g + 1) * J, :].T).astype(bf16) for g in range(4)]
    wvs = [np.ascontiguousarray(Wv[g * J : (g + 1) * J, :].T).astype(bf16) for g in range(4)]

    in_maps = []
    for c in range(N_CORES):
        b, g = c // 4, c % 4
        jsl = slice(g * J, (g + 1) * J)
        in_maps.append(
            {
                "qt": qts[b],
                "wq": wqs[g],
                "wk": wks[g],
                "wv": wvs[g],
                "bq": np.ascontiguousarray(bq[jsl]),
                "bk": np.ascontiguousarray(bk[jsl]),
                "bv": np.ascontiguousarray(bv[jsl]),
            }
        )

    last_result = run_bass_kernel_spmd(nc, in_maps, list(range(N_CORES)))

    full = np.empty((B, S, D), dtype=np.float32)
    for c in range(N_CORES):
        b, g = c // 4, c % 4
        full[b, :, g * J : (g + 1) * J] = last_result.results[c]["out"]
    return full


# revision 60
# speedup vs baseline: 1.0288x; 1.0024x over previous
"""Multi-head self-attention Trainium2 kernel (8 NeuronCores, SPMD).

Problem: B=2, S=2048, D=1024, H=16, Dk=64; torch-style Linear projections
(x @ W.T + b), custom softmax: p = exp(scores/8), attn = p / (sum(p) + 1e-8).

Sharding: 32 (batch, head) pairs over 8 cores -> core c handles batch c//4,
heads [4*(c%4), 4*(c%4)+4). Each core projects only its 256 features of
q/k/v; attention is embarrassingly parallel over (b, h).

All matmul operands are bf16 (same PE rate as fp32r at these shapes, but
half the DMA traffic and LDWEIGHTS time; fp32 accumulation in PSUM keeps
the contractions exact). fp8 q/k via DoubleRow was measured at 2.1e-2
end-to-end and rejected. The PE is the bottleneck engine (~155-165 us of
matmul rows), so the whole kernel is ONE continuous PE instruction stream
with no phase barriers:
  - prologue: only kT[0]/qT[0] chunk 0 (17 matmuls); everything else is
    deadline-paced filler inside the attention loop
  - 128-step attention pipeline over 8 (pair, s-chunk) blocks. Per step:
    2 score matmuls (tile_position row-groups run concurrently in the PE
    array), one exp covering both heads, 2 AV matmuls issued with a 2-step
    LAG so their exp semaphores have long fired when they reach the head
    of the in-order PE queue. Filler pops BETWEEN the exp and the AVs so
    the PE chews projections instead of stalling.
  - filler: remaining projections (kT[0] c1-3, all of v, qT[0] c1-3, all
    of pair 1) with per-unit deadlines derived from first-use steps; the
    x0/x1 PSUM banks rotate between accumulation chains. v s-tile pairs
    pack two 256-col accumulation groups into one 2KB PSUM bank (the
    second group starts on still-pending-zero bytes).
  - exp split across engines: 11/16 t-steps on ACT (table exp, bf16 out),
    5/16 on DVE via a one-instruction Schraudolph in the bf16 bit domain
    (bits16 = trunc(scores*A + B) via f32->u16 convert-on-write, bitcast
    to bf16; ~3% pointwise, mostly cancelled by the sum-normalization;
    8.4e-3 end-to-end vs the 2e-2 budget)
  - DMA layout: the scalar sequencer's issue backlog would gate the first
    exp (DIRECT2D issues are ~0.6us each, in-order with engine ops), so
    qt rides sync (chunk 1 split across both queues), weights ride
    scalar, wv/biases ride gpsimd SWDGE
  - finalize per block: ctx [65,512] PSUM -> SBUF copies (split DVE/ACT),
    per-128-col PE transpose, [128,1] DVE reciprocal of the transposed
    denominator column, one DVE scalar_tensor_tensor out = ctx/den + bv;
    each out tile's column-halves DMA out as soon as their two pieces
    land (the pair-0 half leaves ~40us before the pair-1 half)

Output per core: [2048, 256] fp32 -> host concatenates features per batch.
"""

import sys

sys.path.insert(0, "/opt/trn_rl_repo")

from collections import deque
from contextlib import ExitStack

import ml_dtypes
import numpy as np

import concourse.bass as bass
import concourse.tile as tile
from concourse import bacc, mybir
from concourse.bass_utils import run_bass_kernel_spmd
from concourse.masks import make_identity

F32 = mybir.dt.float32
F32R = mybir.dt.float32r
BF16 = mybir.dt.bfloat16
U16 = mybir.dt.uint16

# Schraudolph exp on DVE, in the bf16 bit domain (the BIR verifier forbids
# int-typed producers feeding fp32r matmuls; bf16 has no such rule):
# exp(g/8) ~= bitcast_bf16(trunc(g*A + B)) via the f32->u16 convert-on-write.
# A = 2^7*log2(e)/8; the down-bias C=5.1 minimizes max pointwise rel err.
SCH_A = float(np.float32((1 << 7) / (8.0 * np.log(2.0))))
SCH_B = float(np.float32((127 << 7) - 5.1))
# t-steps per 16-step block whose exp runs on DVE (rest on ACT)
SCH_STEPS = frozenset({2, 5, 8, 11, 14})

S = 2048  # sequence length
D = 1024  # d_model
J = 256  # features per core (4 heads x 64)
NKT = 8  # k-tiles of the d_model contraction
NSC = 4  # s-chunks of 512
NTT = 16  # t-tiles of 128
N_CORES = 8

_cached_nc = None
last_result = None  # BassKernelResults of the most recent run (for test.py)


def _round_fp32r(x: np.ndarray) -> np.ndarray:
    """Round fp32 to fp32r (keep 11 mantissa bits, round to nearest even)."""
    u = np.ascontiguousarray(x, dtype=np.float32).view(np.uint32)
    r = (u.astype(np.uint64) + 0x7FF + ((u >> 12) & 1)) & 0xFFFFF000
    return r.astype(np.uint32).view(np.float32)


def _build():
    nc = bacc.Bacc(None, target_bir_lowering=False)

    qt = nc.dram_tensor("qt", [D, S], BF16, kind="ExternalInput")
    wq = nc.dram_tensor("wq", [D, J], BF16, kind="ExternalInput")
    wk = nc.dram_tensor("wk", [D, J], BF16, kind="ExternalInput")
    wv = nc.dram_tensor("wv", [D, J], BF16, kind="ExternalInput")
    bq = nc.dram_tensor("bq", [J], F32, kind="ExternalInput")
    bk = nc.dram_tensor("bk", [J], F32, kind="ExternalInput")
    bv = nc.dram_tensor("bv", [J], F32, kind="ExternalInput")
    out = nc.dram_tensor("out", [S, J], F32, kind="ExternalOutput")

    with tile.TileContext(nc) as tc, ExitStack() as ctx:
        wts = ctx.enter_context(tc.tile_pool(name="wts", bufs=1))
        qkp = ctx.enter_context(tc.tile_pool(name="qkp", bufs=1))
        vxp = ctx.enter_context(tc.tile_pool(name="vxp", bufs=1))
        bp = ctx.enter_context(tc.tile_pool(name="bp", bufs=1))
        cxp = ctx.enter_context(tc.tile_pool(name="cxp", bufs=12))
        pTp = ctx.enter_context(tc.tile_pool(name="pTp", bufs=6))
        outp = ctx.enter_context(tc.tile_pool(name="outp", bufs=1))
        qtcp = ctx.enter_context(tc.tile_pool(name="qtc", bufs=1))
        rp = ctx.enter_context(tc.tile_pool(name="rp", bufs=8))
        aps = ctx.enter_context(tc.tile_pool(name="aps", bufs=1, space="PSUM"))
        p1b = ctx.enter_context(tc.tile_pool(name="p1b", bufs=1, space="PSUM"))

        # Weights: 8 k-tiles each of [128, 256], k-major and split across the
        # HWDGE (sync/scalar) / SWDGE (gpsimd) queues, interleaved with the
        # first s-chunk of QT so the first projection matmuls start early
        wq_t = [
            wts.tile([128, J], BF16, name=f"wq{k}", tag=f"wq{k}") for k in range(NKT)
        ]
        wk_t = [
            wts.tile([128, J], BF16, name=f"wk{k}", tag=f"wk{k}") for k in range(NKT)
        ]
        wv_t = [
            wts.tile([128, J], BF16, name=f"wv{k}", tag=f"wv{k}") for k in range(NKT)
        ]
        qtc = [
            qtcp.tile([128, NKT, 512], BF16, name=f"qtc{c}", tag=f"qtc{c}")
            for c in range(NSC)
        ]
        # qtc chunk k-tiles alternate sync/vector HWDGE queues so the first
        # k-tiles land fast; wk streams ahead of wq on scalar (the k chain
        # runs first); wv + biases go via gpsimd SWDGE
        # ident is built FIRST on gpsimd (before its DMA-issue backlog) so
        # the PE warm-up transposes can start at ~7us
        ident = bp.tile([128, 128], F32, name="ident")
        make_identity(nc, ident[:])
        identb = bp.tile([128, 128], BF16, name="identb")
        nc.gpsimd.memset(identb[:], 0.0)
        nc.vector.tensor_copy(identb[:], ident[:])

        # ALL qtc issues on sync: the scalar sequencer must stay shallow --
        # its DMA-issue backlog (0.6us each, in-order with engine ops) would
        # otherwise gate the first exp
        for k in range(NKT):
            ksl = slice(k * 128, (k + 1) * 128)
            nc.sync.dma_start(qtc[0][:, k, :], qt[ksl, 0:512])
            nc.scalar.dma_start(wk_t[k][:], wk[ksl, :])
        for k in range(NKT):
            ksl = slice(k * 128, (k + 1) * 128)
            nc.scalar.dma_start(wq_t[k][:], wq[ksl, :])
            nc.gpsimd.dma_start(wv_t[k][:], wv[ksl, :])
        # chunk 1 is needed soonest (attention t=4..7 plus v s-tiles 4-7):
        # split it across both queues; chunks 2-3 follow on sync
        for k in range(NKT):
            eng = nc.sync if k % 2 == 0 else nc.scalar
            eng.dma_start(qtc[1][:, k, :], qt[k * 128 : (k + 1) * 128, 512:1024])
        for c in range(2, NSC):
            s0 = c * 512
            for k in range(NKT):
                nc.sync.dma_start(
                    qtc[c][:, k, :], qt[k * 128 : (k + 1) * 128, s0 : s0 + 512]
                )

        # Biases: bq/bk as per-partition scalars [128, 2]; bv broadcast [128, 256]
        bq_t = bp.tile([128, 2], F32, name="bqt")
        nc.gpsimd.dma_start(bq_t[:], bq.rearrange("(m p) -> p m", p=128))
        bk_t = bp.tile([128, 2], F32, name="bkt")
        nc.gpsimd.dma_start(bk_t[:], bk.rearrange("(m p) -> p m", p=128))
        bv_t = bp.tile([128, J], F32, name="bvt")
        bvap = bv[:]
        bv_bcast = bass.AP(
            tensor=bvap.tensor, offset=bvap.offset, ap=[[0, 128], [1, J]]
        )
        nc.gpsimd.dma_start(bv_t[:], bv_bcast)

        scratch = bp.tile([128, 1], F32, name="scratch")

        # Persistent projected tensors
        qT = [qkp.tile([128, S], BF16, name=f"qT{m}", tag=f"qT{m}") for m in range(2)]
        kT = [qkp.tile([128, S], BF16, name=f"kT{m}", tag=f"kT{m}") for m in range(2)]
        v_ext = []
        for t in range(NTT):
            vt = vxp.tile([128, 4, 65], BF16, name=f"vx{t}", tag=f"vx{t}")
            # DVE memsets (it is idle during the prologue; gpsimd's queue is
            # busy streaming wv and would delay the first v epilogues)
            nc.vector.memset(vt[:], 1.0)  # ones col [:, h, 64] survives
            v_ext.append(vt)
        out_tiles = [
            outp.tile([128, J], F32, name=f"ot{b}", tag=f"ot{b}") for b in range(16)
        ]

        # pre-load the ACT exp table (it only needs bq_t) so the first
        # attention exp doesn't pay the ~2.7us table-load stall
        nc.scalar.activation(
            scratch[:], bq_t[:, 0:1], mybir.ActivationFunctionType.Exp, scale=0.0
        )

        # ---------- projection instruction chains ----------
        xrot = [0]

        def xtile(shape=(128, 512)):
            t_ = p1b.tile(list(shape), F32, name="px", tag=f"x{xrot[0] % 2}")
            xrot[0] += 1
            return t_

        def emit_qk_chain(pair, c, which, korder=None):
            """8 matmuls + bias epilogue for qT/kT[pair][:, c*512:(c+1)*512]."""
            st = {}
            w_t = wq_t if which == "q" else wk_t
            dst = (qT if which == "q" else kT)[pair]
            b_t = bq_t if which == "q" else bk_t
            ks = list(range(NKT)) if korder is None else korder

            def mm(k):
                def f():
                    if k == ks[0]:
                        st["t"] = xtile()
                    nc.tensor.matmul(
                        st["t"][:],
                        w_t[k][:, pair * 128 : (pair + 1) * 128],
                        qtc[c][:, k, :],
                        start=(k == ks[0]),
                        stop=(k == ks[-1]),
                    )
                return f

            def epi():
                s0 = c * 512
                nc.vector.tensor_scalar_add(
                    dst[:, s0 : s0 + 512], st.pop("t")[:], b_t[:, pair : pair + 1]
                )

            return [mm(k) for k in ks] + [epi]

        def emit_pv_chain(c, half):
            """v for s-tiles (4c+2*half, 4c+2*half+1): 16 matmuls packed into
            one [128,512] PSUM bank (two 256-col accumulation groups; the
            second group starts on still-pending-zero bytes), + 2 epilogues."""
            st = {}
            thunks = []

            def mm(k, sub):
                def f():
                    if k == 0 and sub == 0:
                        st["t"] = xtile()
                    i = 2 * half + sub
                    nc.tensor.matmul(
                        st["t"][:, sub * 256 : (sub + 1) * 256],
                        qtc[c][:, k, i * 128 : (i + 1) * 128],
                        wv_t[k][:],
                        start=(k == 0 and sub == 0),
                        stop=(k == NKT - 1 and sub == 1),
                        skip_group_check=True,
                    )
                return f

            for sub in range(2):
                for k in range(NKT):
                    thunks.append(mm(k, sub))

            def epi(sub):
                def f():
                    i = 2 * half + sub
                    src = st["t"][:, sub * 256 : (sub + 1) * 256]
                    nc.vector.tensor_copy(
                        v_ext[c * 4 + i][:, :, 0:64],
                        src.rearrange("p (h d) -> p h d", h=4),
                    )
                    if sub == 1:
                        st.pop("t")
                return f

            thunks += [epi(0), epi(1)]
            return thunks

        # ---------- prologue: minimum prefix (kT[0]/qT[0] chunk 0) ----------
        for th in emit_qk_chain(0, 0, "k") + emit_qk_chain(0, 0, "q"):
            th()

        # ---------- filler: remaining projections, deadline-paced ----------
        chains = [
            (emit_pv_chain(0, 0), 0),
            (emit_pv_chain(0, 1), 1),
            (emit_qk_chain(0, 1, "k"), 3),
            (emit_pv_chain(1, 0), 4),
            (emit_pv_chain(1, 1), 5),
            (emit_qk_chain(0, 2, "k"), 7),
            (emit_pv_chain(2, 0), 8),
            (emit_pv_chain(2, 1), 9),
            (emit_qk_chain(0, 3, "k"), 11),
            (emit_pv_chain(3, 0), 12),
            (emit_pv_chain(3, 1), 13),
            (emit_qk_chain(0, 1, "q"), 15),
            (emit_qk_chain(0, 2, "q"), 30),
            (emit_qk_chain(0, 3, "q"), 42),
            (emit_qk_chain(1, 0, "k"), 48),
            (emit_qk_chain(1, 0, "q"), 54),
            (emit_qk_chain(1, 1, "k"), 60),
            (emit_qk_chain(1, 1, "q"), 64),
            (emit_qk_chain(1, 2, "k"), 68),
            (emit_qk_chain(1, 2, "q"), 71),
            (emit_qk_chain(1, 3, "k"), 74),
            (emit_qk_chain(1, 3, "q"), 77),
        ]
        work = deque()
        prev_dl = 0
        for thunks, dl in chains:
            n = len(thunks)
            for j, th in enumerate(thunks):
                work.append((prev_dl + (dl - prev_dl) * (j + 1) / n, th))
            prev_dl = dl

        # ---------- attention: one continuous 128-step pipeline ----------
        pieces = deque()
        done_cnt = {}
        piece_tags = [("x0", "x1")]

        def piece(cs_tile, sc, h, pi):
            def f():
                tags = piece_tags[0]
                tag = tags[xrot[0] % len(tags)]
                xrot[0] += 1
                if tag.startswith("x"):
                    tp = p1b.tile([128, 65], BF16, name="tp", tag=tag)
                else:
                    tp = aps.tile([128, 65], BF16, name="tp", tag=tag, bufs=2)
                nc.tensor.transpose(
                    tp[:],
                    cs_tile[0:65, pi * 128 : (pi + 1) * 128],
                    identb[0:65, 0:65],
                )
                blk = sc * 4 + pi
                r = rp.tile([128, 1], F32, name="r", tag="r")
                nc.vector.reciprocal(r[:], tp[:, 64:65])
                nc.vector.scalar_tensor_tensor(
                    out=out_tiles[blk][:, h * 64 : (h + 1) * 64],
                    in0=tp[:, 0:64],
                    scalar=r[:],
                    in1=bv_t[:, h * 64 : (h + 1) * 64],
                    op0=mybir.AluOpType.mult,
                    op1=mybir.AluOpType.add,
                )
                key = (blk, h // 2)
                done_cnt[key] = done_cnt.get(key, 0) + 1
                if done_cnt[key] == 2:
                    jsl = slice((h // 2) * 128, (h // 2) * 128 + 128)
                    eng = nc.sync if blk % 2 == 0 else nc.scalar
                    eng.dma_start(
                        out[blk * 128 : (blk + 1) * 128, jsl],
                        out_tiles[blk][:, jsl],
                    )
            return f

        def fill_slot(i):
            popped = False
            while work and work[0][0] <= i:
                work.popleft()[1]()
                popped = True
            if pieces and not work and not popped:
                pieces.popleft()()
                if len(pieces) > 16 and pieces:
                    pieces.popleft()()

        blocks = [(p, sc) for p in range(2) for sc in range(NSC)]
        NB = len(blocks)
        LAG = 2  # AV issued 2 steps behind its exp: the sem has long fired,
        #          so the in-order PE never stalls at the AV queue head
        ctx_ps = {}
        pts = {}
        for i in range(NB * NTT + LAG):
            if i < NB * NTT:
                b, t = divmod(i, NTT)
                pair, sc = blocks[b]
                s0 = sc * 512
                qTt, kTt = qT[pair], kT[pair]
                if t == 0:
                    ctxA = aps.tile([65, 512], F32, name="ctxA", tag="ctx", bufs=2)
                    ctxB = aps.tile([65, 512], F32, name="ctxB", tag="ctx", bufs=2)
                    ctx_ps[b] = (ctxA, ctxB)
                tsl = slice(t * 128, (t + 1) * 128)
                # both heads' scoresT share one 2-bank tile so a single
                # exp instruction covers both
                g = aps.tile([128, 1024], F32, name="g", tag="grp", bufs=2)
                nc.tensor.matmul(
                    g[:, 0:512],
                    kTt[0:64, tsl],
                    qTt[0:64, s0 : s0 + 512],
                    start=True,
                    stop=True,
                    tile_position=(0, 0),
                )
                nc.tensor.matmul(
                    g[:, 512:1024],
                    kTt[64:128, tsl],
                    qTt[64:128, s0 : s0 + 512],
                    start=True,
                    stop=True,
                    tile_position=(64, 0),
                )
                pT_ = pTp.tile([128, 1024], BF16, name="pT_", tag="pT")
                if t in SCH_STEPS:
                    nc.vector.tensor_scalar(
                        out=pT_[:].bitcast(U16),
                        in0=g[:],
                        scalar1=SCH_A,
                        scalar2=SCH_B,
                        op0=mybir.AluOpType.mult,
                        op1=mybir.AluOpType.add,
                    )
                else:
                    nc.scalar.activation(
                        pT_[:], g[:],
                        mybir.ActivationFunctionType.Exp, scale=0.125,
                    )
                pts[i] = pT_
            # filler between exp-issue and the AV that consumes the previous
            # exp: the PE chews projections/transposes instead of stalling at
            # the queue head while ACT finishes
            fill_slot(i)
            if i >= LAG:
                b, t = divmod(i - LAG, NTT)
                pair, sc = blocks[b]
                hA, hB = 2 * pair, 2 * pair + 1
                ctxA, ctxB = ctx_ps[b]
                pT_ = pts.pop(i - LAG)
                st_, sp_ = (t == 0), (t == NTT - 1)
                nc.tensor.matmul(
                    ctxA[:], v_ext[t][:, hA, :], pT_[:, 0:512],
                    start=st_, stop=sp_,
                )
                nc.tensor.matmul(
                    ctxB[:], v_ext[t][:, hB, :], pT_[:, 512:1024],
                    start=st_, stop=sp_,
                )
                if t == NTT - 1:
                    del ctx_ps[b]
                    csA = cxp.tile([65, 512], BF16, name="csA", tag="cs")
                    nc.vector.tensor_copy(csA[:], ctxA[:])
                    csB = cxp.tile([65, 512], BF16, name="csB", tag="cs")
                    nc.scalar.copy(csB[:], ctxB[:])
                    for pi in range(4):
                        pieces.append(piece(csA, sc, hA, pi))
                        pieces.append(piece(csB, sc, hB, pi))

        # drain
        while work:
            work.popleft()[1]()
        piece_tags[0] = ("x0", "x1", "grp", "grp")
        while pieces:
            pieces.popleft()()

    nc.compile()
    return nc


def kernel(Q, Wq, bq, Wk, bk, Wv, bv):
    global _cached_nc, last_result
    Q = np.asarray(Q, dtype=np.float32)
    Wq, Wk, Wv = (np.asarray(w, dtype=np.float32) for w in (Wq, Wk, Wv))
    bq, bk, bv = (np.asarray(b, dtype=np.float32) for b in (bq, bk, bv))
    B = Q.shape[0]
    assert Q.shape == (B, S, D) and B * 4 == N_CORES

    if _cached_nc is None:
        _cached_nc = _build()
    nc = _cached_nc

    # host-side shard prep (bf16 inputs: full PE rate, half the DMA traffic)
    bf16 = ml_dtypes.bfloat16
    qts = [np.ascontiguousarray(Q[b].T).astype(bf16) for b in range(B)]
    wqs = [np.ascontiguousarray(Wq[g * J : (g + 1) * J, :].T).astype(bf16) for g in range(4)]
    wks = [np.ascontiguousarray(Wk[g * J : (g + 1) * J, :].T).astype(bf16) for g in range(4)]
    wvs = [np.ascontiguousarray(Wv[g * J : (g + 1) * J, :].T).astype(bf16) for g in range(4)]

    in_maps = []
    for c in range(N_CORES):
        b, g = c // 4, c % 4
        jsl = slice(g * J, (g + 1) * J)
        in_maps.append(
            {
                "qt": qts[b],
                "wq": wqs[g],
                "wk": wks[g],
                "wv": wvs[g],
                "bq": np.ascontiguousarray(bq[jsl]),
                "bk": np.ascontiguousarray(bk[jsl]),
                "bv": np.ascontiguousarray(bv[jsl]),
            }
        )

    last_result = run_bass_kernel_spmd(nc, in_maps, list(range(N_CORES)))

    full = np.empty((B, S, D), dtype=np.float32)
    for c in range(N_CORES):
        b, g = c // 4, c % 4
        full[b, :, g * J : (g + 1) * J] = last_result.results[c]["out"]
    return full
